# revision 2
# baseline (speedup 1.0000x reference)
"""GCN message-passing kernel for trn2 (8 NeuronCores) — v2.

Architecture (vs the v1 baseline):
  - Node rows are tile-sharded: tile t (128 rows) owned by core t%8.
    All gather sources (x0 and the hop-1 AllGather output) live in one
    PERMUTED row layout (section-major, rank-major within section), so
    both hops share a single gather/one-hot structure.
  - Each hop: per (sg of <=8 dst tiles, 14336-row window) run, gather
    x[col] rows (fp16, [edge, feat] layout) via dma_gather; scale rows
    by edge values (one batched DVE op per sg via broadcast APs); build
    pure is_equal one-hots for all slots of the sg in one batched DVE
    op; segment-sum via PE matmuls in block-major order into per-tile
    PSUM accumulators y^T [feat, dst]; dense W matmul + bias; per-tile
    l2 norms via PE ones-matmul; transpose back to row layout.
  - Outputs: each core writes ONLY the l2-normalized rows of its own
    shard, densely, for each of the 3 hops ([3, 12544, 128] fp32).
    The host expands pos/neg pair streams by indexing into these
    (pure permutation/duplication; all float math stays on device).
  - Hop-1 x is published via 4 per-section AllGathers issued as soon
    as each section's tiles finish, overlapping the remaining sgs and
    hop-2's early gathers. Hop 2 needs no AllGather at all.
"""
import os
import sys

sys.path.insert(0, "/opt/trn_rl_repo")

import numpy as np

N = 100000
D = 128
P = 128
NCORES = 8
NT_G = 784              # global tiles
NT_L = 98               # tiles per core
SHARD = NT_L * P        # 12544
NPAD = NT_G * P         # 100352
WIN = 14336
NWIN = 7
SEC_TILES = [224, 224, 224, 112]
SEC_BASE_T = [0, 224, 448, 672]
SEC_ROWBASE = [0, 28672, 57344, 86016]
LOC_PER_SEC = [28, 28, 28, 14]
LOC_BASE = [0, 28, 56, 84]
# sgs: (local_tile_start, ntiles); all within one section
SG_LIST = [(0, 8), (8, 8), (16, 8), (24, 4),
           (28, 8), (36, 8), (44, 8), (52, 4),
           (56, 8), (64, 8), (72, 8), (80, 4),
           (84, 8), (92, 6)]
NSG = len(SG_LIST)
# last sg index per section (after which that section's AG can fire)
SEC_LAST_SG = [3, 7, 11, 13]
E_PAIR = 50000

_CACHE = {}
LAST_RESULTS = None


def _ceil(a, b):
    return -(-a // b)


def _pack_idx(idx_arr, cap):
    """Pack idx list (len<=cap*128) into the [128, cap*8] wrapped+replicated
    int16 layout dma_gather expects. Pads with 0."""
    n = cap * P
    buf = np.zeros(n, np.int16)
    buf[: len(idx_arr)] = idx_arr.astype(np.int16)
    blk = buf.reshape(n // 16, 16).T
    return np.tile(blk, (8, 1))


def _tile_sec(t):
    return np.minimum(t // 224, 3)


def _perm_pos(r):
    """Natural row -> position in the permuted (section, rank, tile) layout."""
    r = np.asarray(r, np.int64)
    t = r >> 7
    q = _tile_sec(t)
    c = t % NCORES
    j = (t - np.take(SEC_BASE_T, q)) // NCORES
    return (np.take(SEC_ROWBASE, q)
            + (c * np.take(LOC_PER_SEC, q) + j) * P + (r & 127))


def _local_tile(t):
    """Global tile -> local tile index on its owner core."""
    t = np.asarray(t, np.int64)
    q = _tile_sec(t)
    return np.take(LOC_BASE, q) + (t - np.take(SEC_BASE_T, q)) // NCORES


def _prep(edge_row, edge_col, edge_val):
    """Per-core gather/one-hot metadata, shared by both hops."""
    er = edge_row.astype(np.int64)
    ec = edge_col.astype(np.int64)
    t_dst = er >> 7
    owner = t_dst % NCORES
    ltile = _local_tile(t_dst)
    slot = er & 127
    pos = _perm_pos(ec)
    win = pos // WIN
    gpos = pos % WIN

    sg_of_lt = np.zeros(NT_L, np.int64)
    for si, (lt0, nt) in enumerate(SG_LIST):
        sg_of_lt[lt0: lt0 + nt] = si
    sg = sg_of_lt[ltile]

    per_core = []
    run_counts = np.zeros((NCORES, NSG * NWIN), np.int64)
    for c in range(NCORES):
        m = owner == c
        order = np.lexsort((gpos[m], ltile[m], win[m], sg[m]))
        d = dict(ltile=ltile[m][order], slot=slot[m][order],
                 val=edge_val[m][order], win=win[m][order],
                 sg=sg[m][order], gpos=gpos[m][order])
        key = d["sg"] * NWIN + d["win"]
        run_counts[c] = np.bincount(key, minlength=NSG * NWIN)
        per_core.append(d)
    run_starts = np.zeros((NCORES, NSG * NWIN + 1), np.int64)
    run_starts[:, 1:] = np.cumsum(run_counts, axis=1)

    cap = np.array([_ceil(int(run_counts[:, k].max()), P)
                    for k in range(NSG * NWIN)], np.int64)

    # block-major mm slots: per run, per block, union of local-in-sg tiles
    # mm_slots[s] = list of (block_within_sg, tile_within_sg); blocks are
    # numbered across the sg's 7 windows in window order.
    blk_base = np.zeros(NSG * NWIN, np.int64)  # block idx base within sg
    for s in range(NSG):
        acc = 0
        for w in range(NWIN):
            blk_base[s * NWIN + w] = acc
            acc += int(cap[s * NWIN + w])
    sg_nblk = [int(sum(cap[s * NWIN: (s + 1) * NWIN])) for s in range(NSG)]

    mm_slots = []  # per sg: list of (blk_in_sg, tl) block-major
    for s in range(NSG):
        lt0 = SG_LIST[s][0]
        slots_s = []
        for w in range(NWIN):
            k = s * NWIN + w
            for b in range(int(cap[k])):
                u = set()
                for c in range(NCORES):
                    st, n = run_starts[c, k], run_counts[c, k]
                    lo, hi = b * P, min(b * P + P, int(n))
                    if lo < n:
                        seg = per_core[c]["ltile"][st + lo: st + hi]
                        u.update(np.unique(seg).tolist())
                for t in sorted(u):
                    slots_s.append((int(blk_base[k] + b), int(t - lt0)))
        slots_s.sort(key=lambda x: (x[1], x[0]))
        mm_slots.append(slots_s)
    sg_nmm = [len(x) for x in mm_slots]
    NMM = sum(sg_nmm)
    NBLK = sum(sg_nblk)
    GCOLS = int(cap.sum()) * 8

    gidx_arrs, gslot_arrs, gval_arrs = [], [], []
    for c in range(NCORES):
        d = per_core[c]
        gidx = np.zeros((P, GCOLS), np.int16)
        gslot = np.full((P, NMM), -1.0, np.float32)
        gval = np.zeros((P, NBLK), np.float32)
        gcol = 0
        for s in range(NSG):
            for w in range(NWIN):
                k = s * NWIN + w
                ck = int(cap[k])
                if ck == 0:
                    continue
                st, n = run_starts[c, k], run_counts[c, k]
                gidx[:, gcol: gcol + ck * 8] = _pack_idx(
                    d["gpos"][st: st + n], ck)
                gcol += ck * 8
        # vals, block-major within sg
        blk0 = 0
        for s in range(NSG):
            for w in range(NWIN):
                k = s * NWIN + w
                st, n = run_starts[c, k], run_counts[c, k]
                for b in range(int(cap[k])):
                    lo, hi = b * P, min(b * P + P, int(n))
                    if lo < n:
                        gval[: hi - lo, blk0 + int(blk_base[k]) + b] = \
                            d["val"][st + lo: st + hi]
            blk0 += sg_nblk[s]
        # slots
        mi = 0
        for s in range(NSG):
            lt0 = SG_LIST[s][0]
            # map blk_in_sg -> (k, b)
            kb = {}
            for w in range(NWIN):
                k = s * NWIN + w
                for b in range(int(cap[k])):
                    kb[int(blk_base[k] + b)] = (k, b)
            for (bsg, tl) in mm_slots[s]:
                k, b = kb[bsg]
                st, n = run_starts[c, k], run_counts[c, k]
                lo, hi = b * P, min(b * P + P, int(n))
                if lo < n:
                    seg_t = d["ltile"][st + lo: st + hi]
                    seg_s = d["slot"][st + lo: st + hi]
                    sel = seg_t == (lt0 + tl)
                    col = np.full(P, -1.0, np.float32)
                    col[: hi - lo][sel] = seg_s[sel]
                    gslot[:, mi] = col
                mi += 1
        gidx_arrs.append(gidx)
        gslot_arrs.append(gslot.astype(np.float16))
        gval_arrs.append(gval.astype(np.float16))

    structure = (tuple(cap.tolist()),
                 tuple(x for sl in mm_slots for x in sl),
                 tuple(sg_nmm), tuple(sg_nblk))
    meta = dict(cap=cap, blk_base=blk_base, sg_nblk=sg_nblk, sg_nmm=sg_nmm,
                mm_slots=mm_slots, NMM=NMM, NBLK=NBLK, GCOLS=GCOLS,
                gidx_arrs=gidx_arrs, gslot_arrs=gslot_arrs,
                gval_arrs=gval_arrs)
    return structure, meta


def _build_program(structure, meta):
    import concourse.mybir as mybir
    import concourse.tile as tile
    from concourse import bacc
    from concourse.masks import make_identity

    f16 = mybir.dt.float16
    f32 = mybir.dt.float32
    i16 = mybir.dt.int16

    cap = meta["cap"]
    blk_base = meta["blk_base"]
    sg_nblk = meta["sg_nblk"]
    sg_nmm = meta["sg_nmm"]
    mm_slots = meta["mm_slots"]
    NMM = meta["NMM"]
    NBLK = meta["NBLK"]
    GCOLS = meta["GCOLS"]
    TOTBLK = max(sg_nblk)
    MAXMM = max(sg_nmm)

    nc = bacc.Bacc(None, num_devices=NCORES, num_swdge_queues=4)
    x0p16 = nc.dram_tensor("x0p16", [NPAD, D], f16, kind="ExternalInput")
    x0sh32 = nc.dram_tensor("x0sh32", [SHARD, D], f32, kind="ExternalInput")
    gidx = nc.dram_tensor("gidx", [P, GCOLS], i16, kind="ExternalInput")
    gslot = nc.dram_tensor("gslot", [P, NMM], f16, kind="ExternalInput")
    gval = nc.dram_tensor("gval", [P, NBLK], f16, kind="ExternalInput")
    w1 = nc.dram_tensor("w1", [D, D], f16, kind="ExternalInput")
    w2 = nc.dram_tensor("w2", [D, D], f16, kind="ExternalInput")
    b1 = nc.dram_tensor("b1", [D, 1], f32, kind="ExternalInput")
    b2 = nc.dram_tensor("b2", [D, 1], f32, kind="ExternalInput")
    nrm = nc.dram_tensor("nrm", [3, SHARD, D], f32, kind="ExternalOutput")

    with tile.TileContext(nc) as tc:
        with (
            tc.tile_pool(name="const", bufs=1) as cpool,
            tc.tile_pool(name="meta", bufs=1) as mpool,
            tc.tile_pool(name="gb", bufs=2) as gpool,
            tc.tile_pool(name="oh", bufs=2) as opool,
            tc.tile_pool(name="work", bufs=3) as wpool,
            tc.tile_pool(name="acc", bufs=2) as apool,
            tc.tile_pool(name="psy", bufs=3, space="PSUM") as psy,
            tc.tile_pool(name="psx", bufs=2, space="PSUM") as psx,
            tc.tile_pool(name="psz", bufs=2, space="PSUM") as psz,
            tc.tile_pool(name="dram", bufs=1, space="DRAM") as dram,
        ):
            ident = cpool.tile([P, P], f16)
            make_identity(nc, ident)
            iota_i = cpool.tile([P, P], mybir.dt.int32)
            nc.gpsimd.iota(iota_i, pattern=[[1, P]], base=0,
                           channel_multiplier=0)
            iota_h = cpool.tile([P, P], f16)
            nc.vector.tensor_copy(iota_h, iota_i)
            ones_h = cpool.tile([P, 1], f16)
            nc.vector.memset(ones_h, 1.0)
            w1_t = cpool.tile([P, P], f16)
            nc.sync.dma_start(out=w1_t, in_=w1[:, :])
            w2_t = cpool.tile([P, P], f16)
            nc.sync.dma_start(out=w2_t, in_=w2[:, :])
            b1_t = cpool.tile([P, 1], f32)
            nc.sync.dma_start(out=b1_t, in_=b1[:, :])
            b2_t = cpool.tile([P, 1], f32)
            nc.sync.dma_start(out=b2_t, in_=b2[:, :])
            gidx_t = mpool.tile([P, GCOLS], i16)
            nc.sync.dma_start(out=gidx_t, in_=gidx[:, :])
            gslot_t = mpool.tile([P, NMM], f16)
            nc.sync.dma_start(out=gslot_t, in_=gslot[:, :])
            gval_t = mpool.tile([P, NBLK], f16)
            nc.sync.dma_start(out=gval_t, in_=gval[:, :])

            xsh = dram.tile([SHARD, D], f16)
            xgs = [dram.tile([LOC_PER_SEC[q] * P * NCORES, D], f16,
                             addr_space="Shared", name=f"xg{q}")
                   for q in range(4)]

            # hop 0: l2norm own shard of x0 (fp32), 8 tiles at a time
            for s in range(NSG):
                lt0, nt = SG_LIST[s]
                x0c = wpool.tile([P, 8, D], f32, tag="x0c")
                nc.sync.dma_start(
                    out=x0c[:, :nt, :],
                    in_=x0sh32[lt0 * P: (lt0 + nt) * P, :].rearrange(
                        "(t p) d -> p t d", p=P))
                sq = wpool.tile([P, 8, D], f32, tag="h0sq")
                nc.vector.tensor_tensor(out=sq[:, :nt, :], in0=x0c[:, :nt, :],
                                        in1=x0c[:, :nt, :],
                                        op=mybir.AluOpType.mult)
                ss = wpool.tile([P, 8], f32, tag="h0ss")
                nc.vector.tensor_reduce(out=ss[:, :nt], in_=sq[:, :nt, :],
                                        axis=mybir.AxisListType.X,
                                        op=mybir.AluOpType.add)
                nr = wpool.tile([P, 8], f32, tag="h0nr")
                nc.scalar.sqrt(nr[:, :nt], ss[:, :nt])
                nc.vector.tensor_scalar_max(nr[:, :nt], nr[:, :nt], 1e-12)
                ri = wpool.tile([P, 8], f32, tag="h0ri")
                nc.vector.reciprocal(ri[:, :nt], nr[:, :nt])
                o0 = apool.tile([P, 8, D], f32, tag="h0out")
                nc.vector.tensor_tensor(
                    out=o0[:, :nt, :], in0=x0c[:, :nt, :],
                    in1=ri[:, :nt].rearrange("p (t o) -> p t o",
                                             o=1).to_broadcast([P, nt, D]),
                    op=mybir.AluOpType.mult)
                nc.sync.dma_start(
                    out=nrm[0, lt0 * P: (lt0 + nt) * P, :].rearrange(
                        "(t p) d -> p t d", p=P),
                    in_=o0[:, :nt, :])

            def graph_hop(hop, src, w_t, b_t, write_xsh):
                blk0 = 0
                mm0 = 0
                gcol_of = np.zeros(NSG * NWIN, np.int64)
                acc = 0
                for k in range(NSG * NWIN):
                    gcol_of[k] = acc
                    acc += int(cap[k]) * 8
                for s in range(NSG):
                    lt0, nt = SG_LIST[s]
                    nblk = sg_nblk[s]
                    nmm = sg_nmm[s]
                    gbuf = gpool.tile([P, TOTBLK, P], f16, tag="gbuf")
                    for w in range(NWIN):
                        k = s * NWIN + w
                        ck = int(cap[k])
                        if ck == 0:
                            continue
                        b0 = int(blk_base[k])
                        if isinstance(src, list):
                            q = min(w // 2, 3)
                            w0 = (w - 2 * q) * WIN
                            src_w = src[q][w0: w0 + WIN, :]
                        else:
                            src_w = src[w * WIN: (w + 1) * WIN, :]
                        nc.gpsimd.dma_gather(
                            gbuf[:, b0: b0 + ck, :],
                            src_w,
                            gidx_t[:, gcol_of[k]: gcol_of[k] + ck * 8],
                            num_idxs=ck * P, num_idxs_reg=ck * P,
                            elem_size=D, single_packet=False,
                            queue_num=(s * NWIN + w) % 4,
                        )
                    # scale rows by edge vals (one batched op)
                    nc.vector.tensor_tensor(
                        out=gbuf[:, :nblk, :], in0=gbuf[:, :nblk, :],
                        in1=gval_t[:, blk0: blk0 + nblk].rearrange(
                            "p (b o) -> p b o", o=1).to_broadcast(
                            [P, nblk, P]),
                        op=mybir.AluOpType.mult)
                    # batched one-hot build, in 2 chunks to bound SBUF;
                    # slots are tile-major so each y accumulation is a
                    # single uninterrupted PSUM group.
                    nslots = [0] * nt
                    for (_, tl) in mm_slots[s]:
                        nslots[tl] += 1
                    seen = [0] * nt
                    y_ps = [None] * nt
                    OHC = (MAXMM + 1) // 2
                    oh = None
                    m1c = 0
                    for mi, (bsg, tl) in enumerate(mm_slots[s]):
                        if mi >= m1c:
                            m0c = mi
                            m1c = min(m0c + OHC, nmm)
                            nmmc = m1c - m0c
                            oh = opool.tile([P, OHC, P], f16, tag="oh")
                            nc.vector.tensor_tensor(
                                out=oh[:, :nmmc, :],
                                in0=iota_h[:, :].rearrange(
                                    "p (o c) -> p o c", o=1).to_broadcast(
                                    [P, nmmc, P]),
                                in1=gslot_t[:, mm0 + m0c: mm0 + m1c].rearrange(
                                    "p (m o) -> p m o", o=1).to_broadcast(
                                    [P, nmmc, P]),
                                op=mybir.AluOpType.is_equal)
                        if seen[tl] == 0:
                            y_ps[tl] = psy.tile([P, P], f32, space="PSUM",
                                                tag="y", name="y_ps")
                        seen[tl] += 1
                        nc.tensor.matmul(
                            y_ps[tl], lhsT=gbuf[:, bsg, :],
                            rhs=oh[:, mi - m0c, :],
                            start=(seen[tl] == 1),
                            stop=(seen[tl] == nslots[tl]))
                    # per-tile epilogue
                    xacc = apool.tile([P, 8, D], f16, tag="xacc")
                    oacc = apool.tile([P, 8, D], f32, tag="oacc")
                    for t in range(nt):
                        yT = wpool.tile([P, P], f16, tag="yT")
                        if nslots[t] == 0:
                            nc.vector.memset(yT, 0.0)
                        else:
                            nc.scalar.copy(yT, y_ps[t])
                        xn = psx.tile([P, 512], f32, space="PSUM", tag="xn")
                        x_ps = xn[:, :P]
                        nc.tensor.matmul(x_ps, lhsT=w_t, rhs=yT,
                                         start=True, stop=True)
                        xT = wpool.tile([P, P], f16, tag="xT")
                        nc.scalar.activation(
                            xT, x_ps, mybir.ActivationFunctionType.Identity,
                            bias=b_t[:, :1])
                        sqT = wpool.tile([P, P], f16, tag="sqT")
                        nc.scalar.activation(
                            sqT, x_ps, mybir.ActivationFunctionType.Square,
                            bias=b_t[:, :1])
                        n2_ps = xn[:, P: P + 1]
                        nc.tensor.matmul(n2_ps, lhsT=sqT, rhs=ones_h,
                                         start=True, stop=True)
                        z_ps = psz.tile([P, P], f16, space="PSUM", tag="z")
                        nc.tensor.transpose(z_ps, xT, ident)
                        nc.scalar.copy(xacc[:, t, :], z_ps)
                        nr = wpool.tile([P, 1], f32, tag="nr")
                        nc.scalar.sqrt(nr, n2_ps)
                        nc.vector.tensor_scalar_max(nr, nr, 1e-12)
                        ri = wpool.tile([P, 1], f32, tag="ri")
                        nc.vector.reciprocal(ri, nr)
                        nc.vector.tensor_scalar(
                            out=oacc[:, t, :], in0=xacc[:, t, :],
                            scalar1=ri[:, :1], scalar2=None,
                            op0=mybir.AluOpType.mult)
                    nc.sync.dma_start(
                        out=nrm[hop, lt0 * P: (lt0 + nt) * P, :].rearrange(
                            "(t p) d -> p t d", p=P),
                        in_=oacc[:, :nt, :])
                    if write_xsh:
                        nc.sync.dma_start(
                            out=xsh[lt0 * P: (lt0 + nt) * P, :].rearrange(
                                "(t p) d -> p t d", p=P),
                            in_=xacc[:, :nt, :])
                        for q in range(4):
                            if SEC_LAST_SG[q] == s:
                                r0 = LOC_BASE[q] * P
                                rn = LOC_PER_SEC[q] * P
                                g0 = SEC_ROWBASE[q]
                                gn = rn * NCORES
                                nc.gpsimd.collective_compute(
                                    "AllGather", mybir.AluOpType.bypass,
                                    replica_groups=[list(range(NCORES))],
                                    ins=[xsh[r0: r0 + rn, :].opt()],
                                    outs=[xgs[q][:, :].opt()],
                                )
                    blk0 += nblk
                    mm0 += nmm

            graph_hop(1, x0p16, w1_t, b1_t, True)
            graph_hop(2, xgs, w2_t, b2_t, False)

    nc.compile()
    return nc


def _install_ntff_shim():
    import types
    if "antenv.axon_hooks" in sys.modules:
        return
    mod = types.ModuleType("antenv.axon_hooks")
    mod._hook = None

    def set_axon_ntff_profile_hook(h):
        mod._hook = h

    def get_axon_ntff_profile_hook():
        return mod._hook

    mod.set_axon_ntff_profile_hook = set_axon_ntff_profile_hook
    mod.get_axon_ntff_profile_hook = get_axon_ntff_profile_hook
    sys.modules["antenv.axon_hooks"] = mod
    try:
        from trn_agent_boot.trn_boot import _ntff_profile_via_ctypes
        mod._hook = _ntff_profile_via_ctypes("/opt/axon/libaxon_pjrt.so")
    except Exception:
        mod._hook = None


def kernel(node_emb, attri_emb, W1, b1, W2, b2, edge_val,
           edge_row, edge_col, pos_src, pos_dst, neg_src, neg_dst):
    global LAST_RESULTS
    _install_ntff_shim()
    from concourse.bass_utils import run_bass_kernel_spmd

    structure, meta = _prep(edge_row, edge_col, edge_val)

    import time as _time
    if structure in _CACHE:
        nc = _CACHE[structure]
    else:
        t0 = _time.time()
        nc = _build_program(structure, meta)
        print(f"[kernel] build+schedule: {_time.time() - t0:.1f}s, "
              f"{len(nc.inst_map)} instructions", flush=True)
        _CACHE[structure] = nc

    x0 = np.concatenate([node_emb, attri_emb], axis=0).astype(np.float32)
    x0pad = np.zeros((NPAD, D), np.float32)
    x0pad[:N] = x0
    pp = np.asarray(_perm_pos(np.arange(NPAD)))
    x0perm = np.zeros((NPAD, D), np.float32)
    x0perm[pp] = x0pad
    x0perm16 = x0perm.astype(np.float16)

    in_maps = []
    for c in range(NCORES):
        # core c's shard rows in local-tile order, from the permuted layout
        sl = []
        for q in range(4):
            b = SEC_ROWBASE[q] + c * LOC_PER_SEC[q] * P
            sl.append(x0perm[b: b + LOC_PER_SEC[q] * P])
        in_maps.append({
            "x0p16": x0perm16,
            "x0sh32": np.concatenate(sl, axis=0),
            "gidx": meta["gidx_arrs"][c],
            "gslot": meta["gslot_arrs"][c],
            "gval": meta["gval_arrs"][c],
            "w1": W1.astype(np.float16),
            "w2": W2.astype(np.float16),
            "b1": b1.reshape(D, 1).astype(np.float32),
            "b2": b2.reshape(D, 1).astype(np.float32),
        })

    trace = os.environ.get("BASS_GNN_TRACE", "0") == "1"
    t0 = _time.time()
    res = run_bass_kernel_spmd(nc, in_maps, core_ids=list(range(NCORES)),
                               trace=trace)
    print(f"[kernel] compile+run: {_time.time() - t0:.1f}s", flush=True)
    LAST_RESULTS = res

    # host-side pair expansion from dense normalized shards
    nrm_all = np.stack([res.results[c]["nrm"] for c in range(NCORES)])
    # global row r -> (owner, local row)
    out = np.zeros((4, 3, E_PAIR, D), np.float32)
    streams = [pos_src, pos_dst, neg_src, neg_dst]
    for st, idx in enumerate(streams):
        r = idx.astype(np.int64)
        t = r >> 7
        own = t % NCORES
        lr = np.asarray(_local_tile(t)) * P + (r & 127)
        for h in range(3):
            out[st, h] = nrm_all[own, h, lr]
    return out


# revision 4
# speedup vs baseline: 1.0455x; 1.0455x over previous
"""GCN message-passing kernel for trn2 (8 NeuronCores) — v2.

Architecture (vs the v1 baseline):
  - Node rows are tile-sharded: tile t (128 rows) owned by core t%8.
    All gather sources (x0 and the hop-1 AllGather output) live in one
    PERMUTED row layout (section-major, rank-major within section), so
    both hops share a single gather/one-hot structure.
  - Each hop: per (sg of <=8 dst tiles, 14336-row window) run, gather
    x[col] rows (fp16, [edge, feat] layout) via dma_gather; scale rows
    by edge values (one batched DVE op per sg via broadcast APs); build
    pure is_equal one-hots for all slots of the sg in one batched DVE
    op; segment-sum via PE matmuls in block-major order into per-tile
    PSUM accumulators y^T [feat, dst]; dense W matmul + bias; per-tile
    l2 norms via PE ones-matmul; transpose back to row layout.
  - Outputs: each core writes ONLY the l2-normalized rows of its own
    shard, densely, for each of the 3 hops ([3, 12544, 128] fp32).
    The host expands pos/neg pair streams by indexing into these
    (pure permutation/duplication; all float math stays on device).
  - Hop-1 x is published via 4 per-section AllGathers issued as soon
    as each section's tiles finish, overlapping the remaining sgs and
    hop-2's early gathers. Hop 2 needs no AllGather at all.
"""
import os
import sys

sys.path.insert(0, "/opt/trn_rl_repo")

import numpy as np

N = 100000
D = 128
P = 128
NCORES = 8
NT_G = 784              # global tiles
NT_L = 98               # tiles per core
SHARD = NT_L * P        # 12544
NPAD = NT_G * P         # 100352
WIN = 14336
NWIN = 7
SEC_TILES = [224, 224, 224, 112]
SEC_BASE_T = [0, 224, 448, 672]
SEC_ROWBASE = [0, 28672, 57344, 86016]
LOC_PER_SEC = [28, 28, 28, 14]
LOC_BASE = [0, 28, 56, 84]
# sgs: (local_tile_start, ntiles); all within one section
SG_LIST = [(0, 8), (8, 8), (16, 8), (24, 4),
           (28, 8), (36, 8), (44, 8), (52, 4),
           (56, 8), (64, 8), (72, 8), (80, 4),
           (84, 8), (92, 6)]
NSG = len(SG_LIST)
# last sg index per section (after which that section's AG can fire)
SEC_LAST_SG = [3, 7, 11, 13]
E_PAIR = 50000

_CACHE = {}
LAST_RESULTS = None


def _ceil(a, b):
    return -(-a // b)


def _pack_idx(idx_arr, cap):
    """Pack idx list (len<=cap*128) into the [128, cap*8] wrapped+replicated
    int16 layout dma_gather expects. Pads with 0."""
    n = cap * P
    buf = np.zeros(n, np.int16)
    buf[: len(idx_arr)] = idx_arr.astype(np.int16)
    blk = buf.reshape(n // 16, 16).T
    return np.tile(blk, (8, 1))


def _tile_sec(t):
    return np.minimum(t // 224, 3)


def _perm_pos(r):
    """Natural row -> position in the permuted (section, rank, tile) layout."""
    r = np.asarray(r, np.int64)
    t = r >> 7
    q = _tile_sec(t)
    c = t % NCORES
    j = (t - np.take(SEC_BASE_T, q)) // NCORES
    return (np.take(SEC_ROWBASE, q)
            + (c * np.take(LOC_PER_SEC, q) + j) * P + (r & 127))


def _local_tile(t):
    """Global tile -> local tile index on its owner core."""
    t = np.asarray(t, np.int64)
    q = _tile_sec(t)
    return np.take(LOC_BASE, q) + (t - np.take(SEC_BASE_T, q)) // NCORES


def _prep(edge_row, edge_col, edge_val):
    """Per-core gather/one-hot metadata, shared by both hops."""
    er = edge_row.astype(np.int64)
    ec = edge_col.astype(np.int64)
    t_dst = er >> 7
    owner = t_dst % NCORES
    ltile = _local_tile(t_dst)
    slot = er & 127
    pos = _perm_pos(ec)
    win = pos // WIN
    gpos = pos % WIN

    sg_of_lt = np.zeros(NT_L, np.int64)
    for si, (lt0, nt) in enumerate(SG_LIST):
        sg_of_lt[lt0: lt0 + nt] = si
    sg = sg_of_lt[ltile]

    per_core = []
    run_counts = np.zeros((NCORES, NSG * NWIN), np.int64)
    for c in range(NCORES):
        m = owner == c
        order = np.lexsort((gpos[m], ltile[m], win[m], sg[m]))
        d = dict(ltile=ltile[m][order], slot=slot[m][order],
                 val=edge_val[m][order], win=win[m][order],
                 sg=sg[m][order], gpos=gpos[m][order])
        key = d["sg"] * NWIN + d["win"]
        run_counts[c] = np.bincount(key, minlength=NSG * NWIN)
        per_core.append(d)
    run_starts = np.zeros((NCORES, NSG * NWIN + 1), np.int64)
    run_starts[:, 1:] = np.cumsum(run_counts, axis=1)

    cap = np.array([_ceil(int(run_counts[:, k].max()), P)
                    for k in range(NSG * NWIN)], np.int64)

    # block-major mm slots: per run, per block, union of local-in-sg tiles
    # mm_slots[s] = list of (block_within_sg, tile_within_sg); blocks are
    # numbered across the sg's 7 windows in window order.
    blk_base = np.zeros(NSG * NWIN, np.int64)  # block idx base within sg
    for s in range(NSG):
        acc = 0
        for w in range(NWIN):
            blk_base[s * NWIN + w] = acc
            acc += int(cap[s * NWIN + w])
    sg_nblk = [int(sum(cap[s * NWIN: (s + 1) * NWIN])) for s in range(NSG)]

    mm_slots = []  # per sg: list of (blk_in_sg, tl) block-major
    for s in range(NSG):
        lt0 = SG_LIST[s][0]
        slots_s = []
        for w in range(NWIN):
            k = s * NWIN + w
            for b in range(int(cap[k])):
                u = set()
                for c in range(NCORES):
                    st, n = run_starts[c, k], run_counts[c, k]
                    lo, hi = b * P, min(b * P + P, int(n))
                    if lo < n:
                        seg = per_core[c]["ltile"][st + lo: st + hi]
                        u.update(np.unique(seg).tolist())
                for t in sorted(u):
                    slots_s.append((int(blk_base[k] + b), int(t - lt0)))
        slots_s.sort(key=lambda x: (x[1], x[0]))
        mm_slots.append(slots_s)
    sg_nmm = [len(x) for x in mm_slots]
    NMM = sum(sg_nmm)
    NBLK = sum(sg_nblk)
    GCOLS = int(cap.sum()) * 8

    gidx_arrs, gslot_arrs, gval_arrs = [], [], []
    for c in range(NCORES):
        d = per_core[c]
        gidx = np.zeros((P, GCOLS), np.int16)
        gslot = np.full((P, NMM), -1.0, np.float32)
        gval = np.zeros((P, NBLK), np.float32)
        gcol = 0
        for s in range(NSG):
            for w in range(NWIN):
                k = s * NWIN + w
                ck = int(cap[k])
                if ck == 0:
                    continue
                st, n = run_starts[c, k], run_counts[c, k]
                gidx[:, gcol: gcol + ck * 8] = _pack_idx(
                    d["gpos"][st: st + n], ck)
                gcol += ck * 8
        # vals, block-major within sg
        blk0 = 0
        for s in range(NSG):
            for w in range(NWIN):
                k = s * NWIN + w
                st, n = run_starts[c, k], run_counts[c, k]
                for b in range(int(cap[k])):
                    lo, hi = b * P, min(b * P + P, int(n))
                    if lo < n:
                        gval[: hi - lo, blk0 + int(blk_base[k]) + b] = \
                            d["val"][st + lo: st + hi]
            blk0 += sg_nblk[s]
        # slots
        mi = 0
        for s in range(NSG):
            lt0 = SG_LIST[s][0]
            # map blk_in_sg -> (k, b)
            kb = {}
            for w in range(NWIN):
                k = s * NWIN + w
                for b in range(int(cap[k])):
                    kb[int(blk_base[k] + b)] = (k, b)
            for (bsg, tl) in mm_slots[s]:
                k, b = kb[bsg]
                st, n = run_starts[c, k], run_counts[c, k]
                lo, hi = b * P, min(b * P + P, int(n))
                if lo < n:
                    seg_t = d["ltile"][st + lo: st + hi]
                    seg_s = d["slot"][st + lo: st + hi]
                    sel = seg_t == (lt0 + tl)
                    col = np.full(P, -1.0, np.float32)
                    col[: hi - lo][sel] = seg_s[sel]
                    gslot[:, mi] = col
                mi += 1
        gidx_arrs.append(gidx)
        gslot_arrs.append(gslot.astype(np.float16))
        gval_arrs.append(gval.astype(np.float16))

    structure = (tuple(cap.tolist()),
                 tuple(x for sl in mm_slots for x in sl),
                 tuple(sg_nmm), tuple(sg_nblk))
    meta = dict(cap=cap, blk_base=blk_base, sg_nblk=sg_nblk, sg_nmm=sg_nmm,
                mm_slots=mm_slots, NMM=NMM, NBLK=NBLK, GCOLS=GCOLS,
                gidx_arrs=gidx_arrs, gslot_arrs=gslot_arrs,
                gval_arrs=gval_arrs)
    return structure, meta


def _build_program(structure, meta):
    import concourse.mybir as mybir
    import concourse.tile as tile
    from concourse import bacc
    from concourse.masks import make_identity

    f16 = mybir.dt.float16
    f32 = mybir.dt.float32
    i16 = mybir.dt.int16

    cap = meta["cap"]
    blk_base = meta["blk_base"]
    sg_nblk = meta["sg_nblk"]
    sg_nmm = meta["sg_nmm"]
    mm_slots = meta["mm_slots"]
    NMM = meta["NMM"]
    NBLK = meta["NBLK"]
    GCOLS = meta["GCOLS"]
    TOTBLK = max(sg_nblk)
    MAXMM = max(sg_nmm)

    nc = bacc.Bacc(None, num_devices=NCORES, num_swdge_queues=4)
    x0p16 = nc.dram_tensor("x0p16", [NPAD, D], f16, kind="ExternalInput")
    x0sh32 = nc.dram_tensor("x0sh32", [SHARD, D], f32, kind="ExternalInput")
    gidx = nc.dram_tensor("gidx", [P, GCOLS], i16, kind="ExternalInput")
    gslot = nc.dram_tensor("gslot", [P, NMM], f16, kind="ExternalInput")
    gval = nc.dram_tensor("gval", [P, NBLK], f16, kind="ExternalInput")
    w1 = nc.dram_tensor("w1", [D, D], f16, kind="ExternalInput")
    w2 = nc.dram_tensor("w2", [D, D], f16, kind="ExternalInput")
    b1 = nc.dram_tensor("b1", [D, 1], f32, kind="ExternalInput")
    b2 = nc.dram_tensor("b2", [D, 1], f32, kind="ExternalInput")
    nrm = nc.dram_tensor("nrm", [3, SHARD, D], f32, kind="ExternalOutput")

    with tile.TileContext(nc) as tc:
        with (
            tc.tile_pool(name="const", bufs=1) as cpool,
            tc.tile_pool(name="meta", bufs=1) as mpool,
            tc.tile_pool(name="gb", bufs=2) as gpool,
            tc.tile_pool(name="oh", bufs=2) as opool,
            tc.tile_pool(name="work", bufs=3) as wpool,
            tc.tile_pool(name="acc", bufs=2) as apool,
            tc.tile_pool(name="psy", bufs=3, space="PSUM") as psy,
            tc.tile_pool(name="psx", bufs=2, space="PSUM") as psx,
            tc.tile_pool(name="psz", bufs=2, space="PSUM") as psz,
            tc.tile_pool(name="dram", bufs=1, space="DRAM") as dram,
        ):
            ident = cpool.tile([P, P], f16)
            make_identity(nc, ident)
            iota_i = cpool.tile([P, P], mybir.dt.int32)
            nc.gpsimd.iota(iota_i, pattern=[[1, P]], base=0,
                           channel_multiplier=0)
            iota_h = cpool.tile([P, P], f16)
            nc.vector.tensor_copy(iota_h, iota_i)
            ones_h = cpool.tile([P, 1], f16)
            nc.vector.memset(ones_h, 1.0)
            OHC_C = (max(sg_nmm) + 2) // 3
            iota_rep = cpool.tile([P, OHC_C, P], f16)
            nc.vector.tensor_copy(
                iota_rep,
                iota_h[:, :].rearrange("p (o c) -> p o c", o=1).to_broadcast(
                    [P, OHC_C, P]))
            w1_t = cpool.tile([P, P], f16)
            nc.sync.dma_start(out=w1_t, in_=w1[:, :])
            w2_t = cpool.tile([P, P], f16)
            nc.sync.dma_start(out=w2_t, in_=w2[:, :])
            b1_t = cpool.tile([P, 1], f32)
            nc.sync.dma_start(out=b1_t, in_=b1[:, :])
            b2_t = cpool.tile([P, 1], f32)
            nc.sync.dma_start(out=b2_t, in_=b2[:, :])
            gidx_t = mpool.tile([P, GCOLS], i16)
            nc.sync.dma_start(out=gidx_t, in_=gidx[:, :])
            gslot_t = mpool.tile([P, NMM], f16)
            nc.sync.dma_start(out=gslot_t, in_=gslot[:, :])
            gval_t = mpool.tile([P, NBLK], f16)
            nc.sync.dma_start(out=gval_t, in_=gval[:, :])

            xsh = dram.tile([SHARD, D], f16)
            xgs = [dram.tile([LOC_PER_SEC[q] * P * NCORES, D], f16,
                             addr_space="Shared", name=f"xg{q}")
                   for q in range(4)]

            # hop 0: l2norm own shard of x0 (fp32), 4 tiles at a time
            for lt0 in range(0, NT_L, 4):
                nt = min(4, NT_L - lt0)
                x0c = wpool.tile([P, 4, D], f32, tag="x0c")
                nc.sync.dma_start(
                    out=x0c[:, :nt, :],
                    in_=x0sh32[lt0 * P: (lt0 + nt) * P, :].rearrange(
                        "(t p) d -> p t d", p=P))
                sq = wpool.tile([P, 4, D], f32, tag="h0sq")
                nc.vector.tensor_tensor(out=sq[:, :nt, :], in0=x0c[:, :nt, :],
                                        in1=x0c[:, :nt, :],
                                        op=mybir.AluOpType.mult)
                ss = wpool.tile([P, 4], f32, tag="h0ss")
                nc.vector.tensor_reduce(out=ss[:, :nt], in_=sq[:, :nt, :],
                                        axis=mybir.AxisListType.X,
                                        op=mybir.AluOpType.add)
                nr = wpool.tile([P, 4], f32, tag="h0nr")
                nc.scalar.sqrt(nr[:, :nt], ss[:, :nt])
                nc.vector.tensor_scalar_max(nr[:, :nt], nr[:, :nt], 1e-12)
                ri = wpool.tile([P, 4], f32, tag="h0ri")
                nc.vector.reciprocal(ri[:, :nt], nr[:, :nt])
                o0 = apool.tile([P, 4, D], f32, tag="h0out")
                nc.vector.tensor_tensor(
                    out=o0[:, :nt, :], in0=x0c[:, :nt, :],
                    in1=ri[:, :nt].rearrange("p (t o) -> p t o",
                                             o=1).to_broadcast([P, nt, D]),
                    op=mybir.AluOpType.mult)
                nc.sync.dma_start(
                    out=nrm[0, lt0 * P: (lt0 + nt) * P, :].rearrange(
                        "(t p) d -> p t d", p=P),
                    in_=o0[:, :nt, :])

            def graph_hop(hop, src, w_t, b_t, write_xsh):
                blk0 = 0
                mm0 = 0
                gcol_of = np.zeros(NSG * NWIN, np.int64)
                acc = 0
                for k in range(NSG * NWIN):
                    gcol_of[k] = acc
                    acc += int(cap[k]) * 8
                for s in range(NSG):
                    lt0, nt = SG_LIST[s]
                    nblk = sg_nblk[s]
                    nmm = sg_nmm[s]
                    gbuf = gpool.tile([P, TOTBLK, P], f16, tag="gbuf")
                    for w in range(NWIN):
                        k = s * NWIN + w
                        ck = int(cap[k])
                        if ck == 0:
                            continue
                        b0 = int(blk_base[k])
                        if isinstance(src, list):
                            q = min(w // 2, 3)
                            w0 = (w - 2 * q) * WIN
                            src_w = src[q][w0: w0 + WIN, :]
                        else:
                            src_w = src[w * WIN: (w + 1) * WIN, :]
                        nc.gpsimd.dma_gather(
                            gbuf[:, b0: b0 + ck, :],
                            src_w,
                            gidx_t[:, gcol_of[k]: gcol_of[k] + ck * 8],
                            num_idxs=ck * P, num_idxs_reg=ck * P,
                            elem_size=D, single_packet=False,
                            queue_num=1 + (s * NWIN + w) % 3,
                        )
                    # scale rows by edge vals (one batched op)
                    nc.vector.tensor_tensor(
                        out=gbuf[:, :nblk, :], in0=gbuf[:, :nblk, :],
                        in1=gval_t[:, blk0: blk0 + nblk].rearrange(
                            "p (b o) -> p b o", o=1).to_broadcast(
                            [P, nblk, P]),
                        op=mybir.AluOpType.mult)
                    # batched one-hot build, in 2 chunks to bound SBUF;
                    # slots are tile-major so each y accumulation is a
                    # single uninterrupted PSUM group.
                    nslots = [0] * nt
                    for (_, tl) in mm_slots[s]:
                        nslots[tl] += 1
                    seen = [0] * nt
                    y_ps = [None] * nt
                    OHC = (MAXMM + 2) // 3
                    oh = None
                    m1c = 0
                    for mi, (bsg, tl) in enumerate(mm_slots[s]):
                        if mi >= m1c:
                            m0c = mi
                            m1c = min(m0c + OHC, nmm)
                            nmmc = m1c - m0c
                            oh = opool.tile([P, OHC, P], f16, tag="oh")
                            nc.vector.tensor_tensor(
                                out=oh[:, :nmmc, :],
                                in0=iota_rep[:, :nmmc, :],
                                in1=gslot_t[:, mm0 + m0c: mm0 + m1c].rearrange(
                                    "p (m o) -> p m o", o=1).to_broadcast(
                                    [P, nmmc, P]),
                                op=mybir.AluOpType.is_equal)
                        if seen[tl] == 0:
                            y_ps[tl] = psy.tile([P, P], f32, space="PSUM",
                                                tag="y", name="y_ps")
                        seen[tl] += 1
                        nc.tensor.matmul(
                            y_ps[tl], lhsT=gbuf[:, bsg, :],
                            rhs=oh[:, mi - m0c, :],
                            start=(seen[tl] == 1),
                            stop=(seen[tl] == nslots[tl]))
                    # per-tile epilogue
                    xacc = apool.tile([P, 8, D], f16, tag="xacc")
                    oacc = apool.tile([P, 8, D], f32, tag="oacc")
                    for t in range(nt):
                        yT = wpool.tile([P, P], f16, tag="yT")
                        if nslots[t] == 0:
                            nc.vector.memset(yT, 0.0)
                        else:
                            nc.scalar.copy(yT, y_ps[t])
                        xn = psx.tile([P, 512], f32, space="PSUM", tag="xn")
                        x_ps = xn[:, :P]
                        nc.tensor.matmul(x_ps, lhsT=w_t, rhs=yT,
                                         start=True, stop=True)
                        xT = wpool.tile([P, P], f16, tag="xT")
                        nc.scalar.activation(
                            xT, x_ps, mybir.ActivationFunctionType.Identity,
                            bias=b_t[:, :1])
                        sqT = wpool.tile([P, P], f16, tag="sqT")
                        nc.scalar.activation(
                            sqT, x_ps, mybir.ActivationFunctionType.Square,
                            bias=b_t[:, :1])
                        n2_ps = xn[:, P: P + 1]
                        nc.tensor.matmul(n2_ps, lhsT=sqT, rhs=ones_h,
                                         start=True, stop=True)
                        z_ps = psz.tile([P, P], f16, space="PSUM", tag="z")
                        nc.tensor.transpose(z_ps, xT, ident)
                        nc.scalar.copy(xacc[:, t, :], z_ps)
                        nr = wpool.tile([P, 1], f32, tag="nr")
                        nc.scalar.sqrt(nr, n2_ps)
                        nc.vector.tensor_scalar_max(nr, nr, 1e-12)
                        ri = wpool.tile([P, 1], f32, tag="ri")
                        nc.vector.reciprocal(ri, nr)
                        nc.scalar.mul(oacc[:, t, :], xacc[:, t, :],
                                      ri[:, :1])
                    nc.sync.dma_start(
                        out=nrm[hop, lt0 * P: (lt0 + nt) * P, :].rearrange(
                            "(t p) d -> p t d", p=P),
                        in_=oacc[:, :nt, :])
                    if write_xsh:
                        nc.sync.dma_start(
                            out=xsh[lt0 * P: (lt0 + nt) * P, :].rearrange(
                                "(t p) d -> p t d", p=P),
                            in_=xacc[:, :nt, :])
                        for q in range(4):
                            if SEC_LAST_SG[q] == s:
                                r0 = LOC_BASE[q] * P
                                rn = LOC_PER_SEC[q] * P
                                g0 = SEC_ROWBASE[q]
                                gn = rn * NCORES
                                nc.gpsimd.collective_compute(
                                    "AllGather", mybir.AluOpType.bypass,
                                    replica_groups=[list(range(NCORES))],
                                    ins=[xsh[r0: r0 + rn, :].opt()],
                                    outs=[xgs[q][:, :].opt()],
                                )
                    blk0 += nblk
                    mm0 += nmm

            graph_hop(1, x0p16, w1_t, b1_t, True)
            graph_hop(2, xgs, w2_t, b2_t, False)

    nc.compile()
    return nc


def _install_ntff_shim():
    import types
    if "antenv.axon_hooks" in sys.modules:
        return
    mod = types.ModuleType("antenv.axon_hooks")
    mod._hook = None

    def set_axon_ntff_profile_hook(h):
        mod._hook = h

    def get_axon_ntff_profile_hook():
        return mod._hook

    mod.set_axon_ntff_profile_hook = set_axon_ntff_profile_hook
    mod.get_axon_ntff_profile_hook = get_axon_ntff_profile_hook
    sys.modules["antenv.axon_hooks"] = mod
    try:
        from trn_agent_boot.trn_boot import _ntff_profile_via_ctypes
        mod._hook = _ntff_profile_via_ctypes("/opt/axon/libaxon_pjrt.so")
    except Exception:
        mod._hook = None


def kernel(node_emb, attri_emb, W1, b1, W2, b2, edge_val,
           edge_row, edge_col, pos_src, pos_dst, neg_src, neg_dst):
    global LAST_RESULTS
    _install_ntff_shim()
    from concourse.bass_utils import run_bass_kernel_spmd

    node_emb, attri_emb, W1, b1, W2, b2 = [
        np.asarray(a) for a in (node_emb, attri_emb, W1, b1, W2, b2)]
    edge_val, edge_row, edge_col = [
        np.asarray(a) for a in (edge_val, edge_row, edge_col)]
    pos_src, pos_dst, neg_src, neg_dst = [
        np.asarray(a) for a in (pos_src, pos_dst, neg_src, neg_dst)]

    structure, meta = _prep(edge_row, edge_col, edge_val)

    import time as _time
    if structure in _CACHE:
        nc = _CACHE[structure]
    else:
        t0 = _time.time()
        nc = _build_program(structure, meta)
        print(f"[kernel] build+schedule: {_time.time() - t0:.1f}s, "
              f"{len(nc.inst_map)} instructions", flush=True)
        _CACHE[structure] = nc

    x0 = np.concatenate([node_emb, attri_emb], axis=0).astype(np.float32)
    x0pad = np.zeros((NPAD, D), np.float32)
    x0pad[:N] = x0
    pp = np.asarray(_perm_pos(np.arange(NPAD)))
    x0perm = np.zeros((NPAD, D), np.float32)
    x0perm[pp] = x0pad
    x0perm16 = x0perm.astype(np.float16)

    in_maps = []
    for c in range(NCORES):
        # core c's shard rows in local-tile order, from the permuted layout
        sl = []
        for q in range(4):
            b = SEC_ROWBASE[q] + c * LOC_PER_SEC[q] * P
            sl.append(x0perm[b: b + LOC_PER_SEC[q] * P])
        in_maps.append({
            "x0p16": x0perm16,
            "x0sh32": np.concatenate(sl, axis=0),
            "gidx": meta["gidx_arrs"][c],
            "gslot": meta["gslot_arrs"][c],
            "gval": meta["gval_arrs"][c],
            "w1": W1.astype(np.float16),
            "w2": W2.astype(np.float16),
            "b1": b1.reshape(D, 1).astype(np.float32),
            "b2": b2.reshape(D, 1).astype(np.float32),
        })

    trace = os.environ.get("BASS_GNN_TRACE", "0") == "1"
    t0 = _time.time()
    res = run_bass_kernel_spmd(nc, in_maps, core_ids=list(range(NCORES)),
                               trace=trace)
    print(f"[kernel] compile+run: {_time.time() - t0:.1f}s", flush=True)
    LAST_RESULTS = res

    # host-side pair expansion from dense normalized shards
    nrm_all = np.stack([res.results[c]["nrm"] for c in range(NCORES)])
    # global row r -> (owner, local row)
    out = np.zeros((4, 3, E_PAIR, D), np.float32)
    streams = [pos_src, pos_dst, neg_src, neg_dst]
    for st, idx in enumerate(streams):
        r = idx.astype(np.int64)
        t = r >> 7
        own = t % NCORES
        lr = np.asarray(_local_tile(t)) * P + (r & 127)
        for h in range(3):
            out[st, h] = nrm_all[own, h, lr]
    return out


# revision 5
# speedup vs baseline: 1.0838x; 1.0367x over previous
"""GCN message-passing kernel for trn2 (8 NeuronCores) — v2.

Architecture (vs the v1 baseline):
  - Node rows are tile-sharded: tile t (128 rows) owned by core t%8.
    All gather sources (x0 and the hop-1 AllGather output) live in one
    PERMUTED row layout (section-major, rank-major within section), so
    both hops share a single gather/one-hot structure.
  - Each hop: per (sg of <=8 dst tiles, 14336-row window) run, gather
    x[col] rows (fp16, [edge, feat] layout) via dma_gather; scale rows
    by edge values (one batched DVE op per sg via broadcast APs); build
    pure is_equal one-hots for all slots of the sg in one batched DVE
    op; segment-sum via PE matmuls in block-major order into per-tile
    PSUM accumulators y^T [feat, dst]; dense W matmul + bias; per-tile
    l2 norms via PE ones-matmul; transpose back to row layout.
  - Outputs: each core writes ONLY the l2-normalized rows of its own
    shard, densely, for each of the 3 hops ([3, 12544, 128] fp32).
    The host expands pos/neg pair streams by indexing into these
    (pure permutation/duplication; all float math stays on device).
  - Hop-1 x is published via 4 per-section AllGathers issued as soon
    as each section's tiles finish, overlapping the remaining sgs and
    hop-2's early gathers. Hop 2 needs no AllGather at all.
"""
import os
import sys

sys.path.insert(0, "/opt/trn_rl_repo")

import numpy as np

N = 100000
D = 128
P = 128
NCORES = 8
NT_G = 784              # global tiles
NT_L = 98               # tiles per core
SHARD = NT_L * P        # 12544
NPAD = NT_G * P         # 100352
WIN = 14336
NWIN = 7
SEC_TILES = [224, 224, 224, 112]
SEC_BASE_T = [0, 224, 448, 672]
SEC_ROWBASE = [0, 28672, 57344, 86016]
LOC_PER_SEC = [28, 28, 28, 14]
LOC_BASE = [0, 28, 56, 84]
# sgs: (local_tile_start, ntiles); all within one section
SG_LIST = [(0, 8), (8, 8), (16, 8), (24, 4),
           (28, 8), (36, 8), (44, 8), (52, 4),
           (56, 8), (64, 8), (72, 8), (80, 4),
           (84, 8), (92, 6)]
NSG = len(SG_LIST)
# last sg index per section (after which that section's AG can fire)
SEC_LAST_SG = [3, 7, 11, 13]
E_PAIR = 50000

_CACHE = {}
LAST_RESULTS = None


def _ceil(a, b):
    return -(-a // b)


def _pack_idx(idx_arr, cap):
    """Pack idx list (len<=cap*128) into the [128, cap*8] wrapped+replicated
    int16 layout dma_gather expects. Pads with 0."""
    n = cap * P
    buf = np.zeros(n, np.int16)
    buf[: len(idx_arr)] = idx_arr.astype(np.int16)
    blk = buf.reshape(n // 16, 16).T
    return np.tile(blk, (8, 1))


def _tile_sec(t):
    return np.minimum(t // 224, 3)


def _perm_pos(r):
    """Natural row -> position in the permuted (section, rank, tile) layout."""
    r = np.asarray(r, np.int64)
    t = r >> 7
    q = _tile_sec(t)
    c = t % NCORES
    j = (t - np.take(SEC_BASE_T, q)) // NCORES
    return (np.take(SEC_ROWBASE, q)
            + (c * np.take(LOC_PER_SEC, q) + j) * P + (r & 127))


def _local_tile(t):
    """Global tile -> local tile index on its owner core."""
    t = np.asarray(t, np.int64)
    q = _tile_sec(t)
    return np.take(LOC_BASE, q) + (t - np.take(SEC_BASE_T, q)) // NCORES


def _prep(edge_row, edge_col, edge_val):
    """Per-core gather/one-hot metadata, shared by both hops."""
    er = edge_row.astype(np.int64)
    ec = edge_col.astype(np.int64)
    t_dst = er >> 7
    owner = t_dst % NCORES
    ltile = _local_tile(t_dst)
    slot = er & 127
    pos = _perm_pos(ec)
    win = pos // WIN
    gpos = pos % WIN

    sg_of_lt = np.zeros(NT_L, np.int64)
    for si, (lt0, nt) in enumerate(SG_LIST):
        sg_of_lt[lt0: lt0 + nt] = si
    sg = sg_of_lt[ltile]

    per_core = []
    run_counts = np.zeros((NCORES, NSG * NWIN), np.int64)
    for c in range(NCORES):
        m = owner == c
        order = np.lexsort((gpos[m], ltile[m], win[m], sg[m]))
        d = dict(ltile=ltile[m][order], slot=slot[m][order],
                 val=edge_val[m][order], win=win[m][order],
                 sg=sg[m][order], gpos=gpos[m][order])
        key = d["sg"] * NWIN + d["win"]
        run_counts[c] = np.bincount(key, minlength=NSG * NWIN)
        per_core.append(d)
    run_starts = np.zeros((NCORES, NSG * NWIN + 1), np.int64)
    run_starts[:, 1:] = np.cumsum(run_counts, axis=1)

    cap = np.array([_ceil(int(run_counts[:, k].max()), P)
                    for k in range(NSG * NWIN)], np.int64)

    # block-major mm slots: per run, per block, union of local-in-sg tiles
    # mm_slots[s] = list of (block_within_sg, tile_within_sg); blocks are
    # numbered across the sg's 7 windows in window order.
    blk_base = np.zeros(NSG * NWIN, np.int64)  # block idx base within sg
    for s in range(NSG):
        acc = 0
        for w in range(NWIN):
            blk_base[s * NWIN + w] = acc
            acc += int(cap[s * NWIN + w])
    sg_nblk = [int(sum(cap[s * NWIN: (s + 1) * NWIN])) for s in range(NSG)]

    mm_slots = []  # per sg: list of (blk_in_sg, tl) block-major
    for s in range(NSG):
        lt0 = SG_LIST[s][0]
        slots_s = []
        for w in range(NWIN):
            k = s * NWIN + w
            for b in range(int(cap[k])):
                u = set()
                for c in range(NCORES):
                    st, n = run_starts[c, k], run_counts[c, k]
                    lo, hi = b * P, min(b * P + P, int(n))
                    if lo < n:
                        seg = per_core[c]["ltile"][st + lo: st + hi]
                        u.update(np.unique(seg).tolist())
                for t in sorted(u):
                    slots_s.append((int(blk_base[k] + b), int(t - lt0)))
        slots_s.sort(key=lambda x: (x[1], x[0]))
        mm_slots.append(slots_s)
    sg_nmm = [len(x) for x in mm_slots]
    NMM = sum(sg_nmm)
    NBLK = sum(sg_nblk)
    GCOLS = int(cap.sum()) * 8

    gidx_arrs, gslot_arrs, gval_arrs = [], [], []
    for c in range(NCORES):
        d = per_core[c]
        gidx = np.zeros((P, GCOLS), np.int16)
        gslot = np.full((P, NMM), -1.0, np.float32)
        gval = np.zeros((P, NBLK), np.float32)
        gcol = 0
        for s in range(NSG):
            for w in range(NWIN):
                k = s * NWIN + w
                ck = int(cap[k])
                if ck == 0:
                    continue
                st, n = run_starts[c, k], run_counts[c, k]
                gidx[:, gcol: gcol + ck * 8] = _pack_idx(
                    d["gpos"][st: st + n], ck)
                gcol += ck * 8
        # vals, block-major within sg
        blk0 = 0
        for s in range(NSG):
            for w in range(NWIN):
                k = s * NWIN + w
                st, n = run_starts[c, k], run_counts[c, k]
                for b in range(int(cap[k])):
                    lo, hi = b * P, min(b * P + P, int(n))
                    if lo < n:
                        gval[: hi - lo, blk0 + int(blk_base[k]) + b] = \
                            d["val"][st + lo: st + hi]
            blk0 += sg_nblk[s]
        # slots
        mi = 0
        for s in range(NSG):
            lt0 = SG_LIST[s][0]
            # map blk_in_sg -> (k, b)
            kb = {}
            for w in range(NWIN):
                k = s * NWIN + w
                for b in range(int(cap[k])):
                    kb[int(blk_base[k] + b)] = (k, b)
            for (bsg, tl) in mm_slots[s]:
                k, b = kb[bsg]
                st, n = run_starts[c, k], run_counts[c, k]
                lo, hi = b * P, min(b * P + P, int(n))
                if lo < n:
                    seg_t = d["ltile"][st + lo: st + hi]
                    seg_s = d["slot"][st + lo: st + hi]
                    sel = seg_t == (lt0 + tl)
                    col = np.full(P, -1.0, np.float32)
                    col[: hi - lo][sel] = seg_s[sel]
                    gslot[:, mi] = col
                mi += 1
        gidx_arrs.append(gidx)
        gslot_arrs.append(gslot.astype(np.float16))
        gval_arrs.append(gval.astype(np.float16))

    structure = (tuple(cap.tolist()),
                 tuple(x for sl in mm_slots for x in sl),
                 tuple(sg_nmm), tuple(sg_nblk))
    meta = dict(cap=cap, blk_base=blk_base, sg_nblk=sg_nblk, sg_nmm=sg_nmm,
                mm_slots=mm_slots, NMM=NMM, NBLK=NBLK, GCOLS=GCOLS,
                gidx_arrs=gidx_arrs, gslot_arrs=gslot_arrs,
                gval_arrs=gval_arrs)
    return structure, meta


def _build_program(structure, meta):
    import concourse.mybir as mybir
    import concourse.tile as tile
    from concourse import bacc
    from concourse.masks import make_identity

    f16 = mybir.dt.float16
    f32 = mybir.dt.float32
    i16 = mybir.dt.int16

    cap = meta["cap"]
    blk_base = meta["blk_base"]
    sg_nblk = meta["sg_nblk"]
    sg_nmm = meta["sg_nmm"]
    mm_slots = meta["mm_slots"]
    NMM = meta["NMM"]
    NBLK = meta["NBLK"]
    GCOLS = meta["GCOLS"]
    TOTBLK = max(sg_nblk)
    MAXMM = max(sg_nmm)

    nc = bacc.Bacc(None, num_devices=NCORES, num_swdge_queues=4)
    x0p16 = nc.dram_tensor("x0p16", [NPAD, D], f16, kind="ExternalInput")
    x0sh32 = nc.dram_tensor("x0sh32", [SHARD, D], f32, kind="ExternalInput")
    gidx = nc.dram_tensor("gidx", [P, GCOLS], i16, kind="ExternalInput")
    gslot = nc.dram_tensor("gslot", [P, NMM], f16, kind="ExternalInput")
    gval = nc.dram_tensor("gval", [P, NBLK], f16, kind="ExternalInput")
    w1 = nc.dram_tensor("w1", [D, D], f16, kind="ExternalInput")
    w2 = nc.dram_tensor("w2", [D, D], f16, kind="ExternalInput")
    b1 = nc.dram_tensor("b1", [D, 1], f32, kind="ExternalInput")
    b2 = nc.dram_tensor("b2", [D, 1], f32, kind="ExternalInput")
    nrm = nc.dram_tensor("nrm", [3, SHARD, D], f32, kind="ExternalOutput")

    with tile.TileContext(nc) as tc:
        with (
            tc.tile_pool(name="const", bufs=1) as cpool,
            tc.tile_pool(name="meta", bufs=1) as mpool,
            tc.tile_pool(name="gb", bufs=2) as gpool,
            tc.tile_pool(name="oh", bufs=3) as opool,
            tc.tile_pool(name="work", bufs=3) as wpool,
            tc.tile_pool(name="acc", bufs=2) as apool,
            tc.tile_pool(name="psy", bufs=3, space="PSUM") as psy,
            tc.tile_pool(name="psx", bufs=2, space="PSUM") as psx,
            tc.tile_pool(name="psz", bufs=3, space="PSUM") as psz,
            tc.tile_pool(name="dram", bufs=1, space="DRAM") as dram,
        ):
            ident = cpool.tile([P, P], f16)
            make_identity(nc, ident)
            iota_i = cpool.tile([P, P], mybir.dt.int32)
            nc.gpsimd.iota(iota_i, pattern=[[1, P]], base=0,
                           channel_multiplier=0)
            iota_h = cpool.tile([P, P], f16)
            nc.vector.tensor_copy(iota_h, iota_i)
            ones_h = cpool.tile([P, 1], f16)
            nc.vector.memset(ones_h, 1.0)
            OHC_C = (max(sg_nmm) + 3) // 4
            iota_rep = cpool.tile([P, OHC_C, P], f16)
            nc.vector.tensor_copy(
                iota_rep,
                iota_h[:, :].rearrange("p (o c) -> p o c", o=1).to_broadcast(
                    [P, OHC_C, P]))
            w1_t = cpool.tile([P, P], f16)
            nc.sync.dma_start(out=w1_t, in_=w1[:, :])
            w2_t = cpool.tile([P, P], f16)
            nc.sync.dma_start(out=w2_t, in_=w2[:, :])
            b1_t = cpool.tile([P, 1], f32)
            nc.sync.dma_start(out=b1_t, in_=b1[:, :])
            b2_t = cpool.tile([P, 1], f32)
            nc.sync.dma_start(out=b2_t, in_=b2[:, :])
            gidx_t = mpool.tile([P, GCOLS], i16)
            nc.sync.dma_start(out=gidx_t, in_=gidx[:, :])
            gslot_t = mpool.tile([P, NMM], f16)
            nc.sync.dma_start(out=gslot_t, in_=gslot[:, :])
            gval_t = mpool.tile([P, NBLK], f16)
            nc.sync.dma_start(out=gval_t, in_=gval[:, :])

            xsh = dram.tile([SHARD, D], f16)
            xgs = [dram.tile([LOC_PER_SEC[q] * P * NCORES, D], f16,
                             addr_space="Shared", name=f"xg{q}")
                   for q in range(4)]

            # hop 0: l2norm own shard of x0 (fp32), 4 tiles at a time
            for lt0 in range(0, NT_L, 4):
                nt = min(4, NT_L - lt0)
                x0c = wpool.tile([P, 4, D], f32, tag="x0c")
                nc.sync.dma_start(
                    out=x0c[:, :nt, :],
                    in_=x0sh32[lt0 * P: (lt0 + nt) * P, :].rearrange(
                        "(t p) d -> p t d", p=P))
                sq = wpool.tile([P, 4, D], f32, tag="h0sq")
                nc.vector.tensor_tensor(out=sq[:, :nt, :], in0=x0c[:, :nt, :],
                                        in1=x0c[:, :nt, :],
                                        op=mybir.AluOpType.mult)
                ss = wpool.tile([P, 4], f32, tag="h0ss")
                nc.vector.tensor_reduce(out=ss[:, :nt], in_=sq[:, :nt, :],
                                        axis=mybir.AxisListType.X,
                                        op=mybir.AluOpType.add)
                nr = wpool.tile([P, 4], f32, tag="h0nr")
                nc.scalar.sqrt(nr[:, :nt], ss[:, :nt])
                nc.vector.tensor_scalar_max(nr[:, :nt], nr[:, :nt], 1e-12)
                ri = wpool.tile([P, 4], f32, tag="h0ri")
                nc.vector.reciprocal(ri[:, :nt], nr[:, :nt])
                o0 = apool.tile([P, 4, D], f32, tag="h0out")
                nc.vector.tensor_tensor(
                    out=o0[:, :nt, :], in0=x0c[:, :nt, :],
                    in1=ri[:, :nt].rearrange("p (t o) -> p t o",
                                             o=1).to_broadcast([P, nt, D]),
                    op=mybir.AluOpType.mult)
                nc.sync.dma_start(
                    out=nrm[0, lt0 * P: (lt0 + nt) * P, :].rearrange(
                        "(t p) d -> p t d", p=P),
                    in_=o0[:, :nt, :])

            def graph_hop(hop, src, w_t, b_t, write_xsh):
                blk0 = 0
                mm0 = 0
                gcol_of = np.zeros(NSG * NWIN, np.int64)
                acc = 0
                for k in range(NSG * NWIN):
                    gcol_of[k] = acc
                    acc += int(cap[k]) * 8
                for s in range(NSG):
                    lt0, nt = SG_LIST[s]
                    nblk = sg_nblk[s]
                    nmm = sg_nmm[s]
                    gbuf = gpool.tile([P, TOTBLK, P], f16, tag="gbuf")
                    for w in range(NWIN):
                        k = s * NWIN + w
                        ck = int(cap[k])
                        if ck == 0:
                            continue
                        b0 = int(blk_base[k])
                        if isinstance(src, list):
                            q = min(w // 2, 3)
                            w0 = (w - 2 * q) * WIN
                            src_w = src[q][w0: w0 + WIN, :]
                        else:
                            src_w = src[w * WIN: (w + 1) * WIN, :]
                        nc.gpsimd.dma_gather(
                            gbuf[:, b0: b0 + ck, :],
                            src_w,
                            gidx_t[:, gcol_of[k]: gcol_of[k] + ck * 8],
                            num_idxs=ck * P, num_idxs_reg=ck * P,
                            elem_size=D, single_packet=False,
                            queue_num=1 + (s * NWIN + w) % 3,
                        )
                    # scale rows by edge vals (one batched op)
                    nc.vector.tensor_tensor(
                        out=gbuf[:, :nblk, :], in0=gbuf[:, :nblk, :],
                        in1=gval_t[:, blk0: blk0 + nblk].rearrange(
                            "p (b o) -> p b o", o=1).to_broadcast(
                            [P, nblk, P]),
                        op=mybir.AluOpType.mult)
                    # batched one-hot build, in 2 chunks to bound SBUF;
                    # slots are tile-major so each y accumulation is a
                    # single uninterrupted PSUM group.
                    nslots = [0] * nt
                    for (_, tl) in mm_slots[s]:
                        nslots[tl] += 1
                    seen = [0] * nt
                    y_ps = [None] * nt
                    OHC = (MAXMM + 3) // 4
                    oh = None
                    m1c = 0
                    for mi, (bsg, tl) in enumerate(mm_slots[s]):
                        if mi >= m1c:
                            m0c = mi
                            m1c = min(m0c + OHC, nmm)
                            nmmc = m1c - m0c
                            oh = opool.tile([P, OHC, P], f16, tag="oh")
                            nc.vector.tensor_tensor(
                                out=oh[:, :nmmc, :],
                                in0=iota_rep[:, :nmmc, :],
                                in1=gslot_t[:, mm0 + m0c: mm0 + m1c].rearrange(
                                    "p (m o) -> p m o", o=1).to_broadcast(
                                    [P, nmmc, P]),
                                op=mybir.AluOpType.is_equal)
                        if seen[tl] == 0:
                            y_ps[tl] = psy.tile([P, P], f32, space="PSUM",
                                                tag="y", name="y_ps")
                        seen[tl] += 1
                        nc.tensor.matmul(
                            y_ps[tl], lhsT=gbuf[:, bsg, :],
                            rhs=oh[:, mi - m0c, :],
                            start=(seen[tl] == 1),
                            stop=(seen[tl] == nslots[tl]))
                    # per-tile epilogue
                    xacc = apool.tile([P, 8, D], f16, tag="xacc")
                    oacc = apool.tile([P, 8, D], f32, tag="oacc")
                    for t in range(nt):
                        yT = wpool.tile([P, P], f16, tag="yT")
                        if nslots[t] == 0:
                            nc.vector.memset(yT, 0.0)
                        else:
                            nc.scalar.copy(yT, y_ps[t])
                        xn = psx.tile([P, 512], f32, space="PSUM", tag="xn")
                        x_ps = xn[:, :P]
                        nc.tensor.matmul(x_ps, lhsT=w_t, rhs=yT,
                                         start=True, stop=True)
                        xT = wpool.tile([P, P], f16, tag="xT")
                        nc.scalar.activation(
                            xT, x_ps, mybir.ActivationFunctionType.Identity,
                            bias=b_t[:, :1])
                        sqT = wpool.tile([P, P], f16, tag="sqT")
                        nc.scalar.activation(
                            sqT, x_ps, mybir.ActivationFunctionType.Square,
                            bias=b_t[:, :1])
                        n2_ps = xn[:, P: P + 1]
                        nc.tensor.matmul(n2_ps, lhsT=sqT, rhs=ones_h,
                                         start=True, stop=True)
                        z_ps = psz.tile([P, P], f16, space="PSUM", tag="z")
                        nc.tensor.transpose(z_ps, xT, ident)
                        nc.scalar.copy(xacc[:, t, :], z_ps)
                        nr = wpool.tile([P, 1], f32, tag="nr")
                        nc.scalar.sqrt(nr, n2_ps)
                        nc.vector.tensor_scalar_max(nr, nr, 1e-12)
                        ri = wpool.tile([P, 1], f32, tag="ri")
                        nc.vector.reciprocal(ri, nr)
                        nc.scalar.mul(oacc[:, t, :], xacc[:, t, :],
                                      ri[:, :1])
                    nc.sync.dma_start(
                        out=nrm[hop, lt0 * P: (lt0 + nt) * P, :].rearrange(
                            "(t p) d -> p t d", p=P),
                        in_=oacc[:, :nt, :])
                    if write_xsh:
                        nc.sync.dma_start(
                            out=xsh[lt0 * P: (lt0 + nt) * P, :].rearrange(
                                "(t p) d -> p t d", p=P),
                            in_=xacc[:, :nt, :])
                        for q in range(4):
                            if SEC_LAST_SG[q] == s:
                                r0 = LOC_BASE[q] * P
                                rn = LOC_PER_SEC[q] * P
                                g0 = SEC_ROWBASE[q]
                                gn = rn * NCORES
                                nc.gpsimd.collective_compute(
                                    "AllGather", mybir.AluOpType.bypass,
                                    replica_groups=[list(range(NCORES))],
                                    ins=[xsh[r0: r0 + rn, :].opt()],
                                    outs=[xgs[q][:, :].opt()],
                                )
                    blk0 += nblk
                    mm0 += nmm

            graph_hop(1, x0p16, w1_t, b1_t, True)
            graph_hop(2, xgs, w2_t, b2_t, False)

    nc.compile()
    return nc


def _install_ntff_shim():
    import types
    if "antenv.axon_hooks" in sys.modules:
        return
    mod = types.ModuleType("antenv.axon_hooks")
    mod._hook = None

    def set_axon_ntff_profile_hook(h):
        mod._hook = h

    def get_axon_ntff_profile_hook():
        return mod._hook

    mod.set_axon_ntff_profile_hook = set_axon_ntff_profile_hook
    mod.get_axon_ntff_profile_hook = get_axon_ntff_profile_hook
    sys.modules["antenv.axon_hooks"] = mod
    try:
        from trn_agent_boot.trn_boot import _ntff_profile_via_ctypes
        mod._hook = _ntff_profile_via_ctypes("/opt/axon/libaxon_pjrt.so")
    except Exception:
        mod._hook = None


def kernel(node_emb, attri_emb, W1, b1, W2, b2, edge_val,
           edge_row, edge_col, pos_src, pos_dst, neg_src, neg_dst):
    global LAST_RESULTS
    _install_ntff_shim()
    from concourse.bass_utils import run_bass_kernel_spmd

    node_emb, attri_emb, W1, b1, W2, b2 = [
        np.asarray(a) for a in (node_emb, attri_emb, W1, b1, W2, b2)]
    edge_val, edge_row, edge_col = [
        np.asarray(a) for a in (edge_val, edge_row, edge_col)]
    pos_src, pos_dst, neg_src, neg_dst = [
        np.asarray(a) for a in (pos_src, pos_dst, neg_src, neg_dst)]

    structure, meta = _prep(edge_row, edge_col, edge_val)

    import time as _time
    if structure in _CACHE:
        nc = _CACHE[structure]
    else:
        t0 = _time.time()
        nc = _build_program(structure, meta)
        print(f"[kernel] build+schedule: {_time.time() - t0:.1f}s, "
              f"{len(nc.inst_map)} instructions", flush=True)
        _CACHE[structure] = nc

    x0 = np.concatenate([node_emb, attri_emb], axis=0).astype(np.float32)
    x0pad = np.zeros((NPAD, D), np.float32)
    x0pad[:N] = x0
    pp = np.asarray(_perm_pos(np.arange(NPAD)))
    x0perm = np.zeros((NPAD, D), np.float32)
    x0perm[pp] = x0pad
    x0perm16 = x0perm.astype(np.float16)

    in_maps = []
    for c in range(NCORES):
        # core c's shard rows in local-tile order, from the permuted layout
        sl = []
        for q in range(4):
            b = SEC_ROWBASE[q] + c * LOC_PER_SEC[q] * P
            sl.append(x0perm[b: b + LOC_PER_SEC[q] * P])
        in_maps.append({
            "x0p16": x0perm16,
            "x0sh32": np.concatenate(sl, axis=0),
            "gidx": meta["gidx_arrs"][c],
            "gslot": meta["gslot_arrs"][c],
            "gval": meta["gval_arrs"][c],
            "w1": W1.astype(np.float16),
            "w2": W2.astype(np.float16),
            "b1": b1.reshape(D, 1).astype(np.float32),
            "b2": b2.reshape(D, 1).astype(np.float32),
        })

    trace = os.environ.get("BASS_GNN_TRACE", "0") == "1"
    t0 = _time.time()
    res = run_bass_kernel_spmd(nc, in_maps, core_ids=list(range(NCORES)),
                               trace=trace)
    print(f"[kernel] compile+run: {_time.time() - t0:.1f}s", flush=True)
    LAST_RESULTS = res

    # host-side pair expansion from dense normalized shards
    nrm_all = np.stack([res.results[c]["nrm"] for c in range(NCORES)])
    # global row r -> (owner, local row)
    out = np.zeros((4, 3, E_PAIR, D), np.float32)
    streams = [pos_src, pos_dst, neg_src, neg_dst]
    for st, idx in enumerate(streams):
        r = idx.astype(np.int64)
        t = r >> 7
        own = t % NCORES
        lr = np.asarray(_local_tile(t)) * P + (r & 127)
        for h in range(3):
            out[st, h] = nrm_all[own, h, lr]
    return out


# revision 6
# speedup vs baseline: 1.1684x; 1.0780x over previous
"""GCN message-passing kernel for trn2 (8 NeuronCores) — v2.

Architecture (vs the v1 baseline):
  - Node rows are tile-sharded: tile t (128 rows) owned by core t%8.
    All gather sources (x0 and the hop-1 AllGather output) live in one
    PERMUTED row layout (section-major, rank-major within section), so
    both hops share a single gather/one-hot structure.
  - Each hop: per (sg of <=8 dst tiles, 14336-row window) run, gather
    x[col] rows (fp16, [edge, feat] layout) via dma_gather; scale rows
    by edge values (one batched DVE op per sg via broadcast APs); build
    pure is_equal one-hots for all slots of the sg in one batched DVE
    op; segment-sum via PE matmuls in block-major order into per-tile
    PSUM accumulators y^T [feat, dst]; dense W matmul + bias; per-tile
    l2 norms via PE ones-matmul; transpose back to row layout.
  - Outputs: each core writes ONLY the l2-normalized rows of its own
    shard, densely, for each of the 3 hops ([3, 12544, 128] fp32).
    The host expands pos/neg pair streams by indexing into these
    (pure permutation/duplication; all float math stays on device).
  - Hop-1 x is published via 4 per-section AllGathers issued as soon
    as each section's tiles finish, overlapping the remaining sgs and
    hop-2's early gathers. Hop 2 needs no AllGather at all.
"""
import os
import sys

sys.path.insert(0, "/opt/trn_rl_repo")

import numpy as np

N = 100000
D = 128
P = 128
NCORES = 8
NT_G = 784              # global tiles
NT_L = 98               # tiles per core
SHARD = NT_L * P        # 12544
NPAD = NT_G * P         # 100352
WIN = 14336
NWIN = 7
SEC_TILES = [224, 224, 224, 112]
SEC_BASE_T = [0, 224, 448, 672]
SEC_ROWBASE = [0, 28672, 57344, 86016]
LOC_PER_SEC = [28, 28, 28, 14]
LOC_BASE = [0, 28, 56, 84]
# sgs: (local_tile_start, ntiles); all within one section
SG_LIST = [(0, 8), (8, 8), (16, 8), (24, 4),
           (28, 8), (36, 8), (44, 8), (52, 4),
           (56, 8), (64, 8), (72, 8), (80, 4),
           (84, 8), (92, 6)]
NSG = len(SG_LIST)
# last sg index per section (after which that section's AG can fire)
SEC_LAST_SG = [3, 7, 11, 13]
E_PAIR = 50000

_CACHE = {}
LAST_RESULTS = None


def _ceil(a, b):
    return -(-a // b)


def _pack_idx(idx_arr, cap):
    """Pack idx list (len<=cap*128) into the [128, cap*8] wrapped+replicated
    int16 layout dma_gather expects. Pads with 0."""
    n = cap * P
    buf = np.zeros(n, np.int16)
    buf[: len(idx_arr)] = idx_arr.astype(np.int16)
    blk = buf.reshape(n // 16, 16).T
    return np.tile(blk, (8, 1))


def _tile_sec(t):
    return np.minimum(t // 224, 3)


def _perm_pos(r):
    """Natural row -> position in the permuted (section, rank, tile) layout."""
    r = np.asarray(r, np.int64)
    t = r >> 7
    q = _tile_sec(t)
    c = t % NCORES
    j = (t - np.take(SEC_BASE_T, q)) // NCORES
    return (np.take(SEC_ROWBASE, q)
            + (c * np.take(LOC_PER_SEC, q) + j) * P + (r & 127))


def _local_tile(t):
    """Global tile -> local tile index on its owner core."""
    t = np.asarray(t, np.int64)
    q = _tile_sec(t)
    return np.take(LOC_BASE, q) + (t - np.take(SEC_BASE_T, q)) // NCORES


def _prep(edge_row, edge_col, edge_val):
    """Per-core gather/one-hot metadata, shared by both hops."""
    er = edge_row.astype(np.int64)
    ec = edge_col.astype(np.int64)
    t_dst = er >> 7
    owner = t_dst % NCORES
    ltile = _local_tile(t_dst)
    slot = er & 127
    pos = _perm_pos(ec)
    win = pos // WIN
    gpos = pos % WIN

    sg_of_lt = np.zeros(NT_L, np.int64)
    for si, (lt0, nt) in enumerate(SG_LIST):
        sg_of_lt[lt0: lt0 + nt] = si
    sg = sg_of_lt[ltile]

    per_core = []
    run_counts = np.zeros((NCORES, NSG * NWIN), np.int64)
    for c in range(NCORES):
        m = owner == c
        order = np.lexsort((gpos[m], ltile[m], win[m], sg[m]))
        d = dict(ltile=ltile[m][order], slot=slot[m][order],
                 val=edge_val[m][order], win=win[m][order],
                 sg=sg[m][order], gpos=gpos[m][order])
        key = d["sg"] * NWIN + d["win"]
        run_counts[c] = np.bincount(key, minlength=NSG * NWIN)
        per_core.append(d)
    run_starts = np.zeros((NCORES, NSG * NWIN + 1), np.int64)
    run_starts[:, 1:] = np.cumsum(run_counts, axis=1)

    cap = np.array([_ceil(int(run_counts[:, k].max()), P)
                    for k in range(NSG * NWIN)], np.int64)

    # block-major mm slots: per run, per block, union of local-in-sg tiles
    # mm_slots[s] = list of (block_within_sg, tile_within_sg); blocks are
    # numbered across the sg's 7 windows in window order.
    blk_base = np.zeros(NSG * NWIN, np.int64)  # block idx base within sg
    for s in range(NSG):
        acc = 0
        for w in range(NWIN):
            blk_base[s * NWIN + w] = acc
            acc += int(cap[s * NWIN + w])
    sg_nblk = [int(sum(cap[s * NWIN: (s + 1) * NWIN])) for s in range(NSG)]

    mm_slots = []  # per sg: list of (blk_in_sg, tl) block-major
    for s in range(NSG):
        lt0 = SG_LIST[s][0]
        slots_s = []
        for w in range(NWIN):
            k = s * NWIN + w
            for b in range(int(cap[k])):
                u = set()
                for c in range(NCORES):
                    st, n = run_starts[c, k], run_counts[c, k]
                    lo, hi = b * P, min(b * P + P, int(n))
                    if lo < n:
                        seg = per_core[c]["ltile"][st + lo: st + hi]
                        u.update(np.unique(seg).tolist())
                for t in sorted(u):
                    slots_s.append((int(blk_base[k] + b), int(t - lt0)))
        slots_s.sort(key=lambda x: (x[1], x[0]))
        mm_slots.append(slots_s)
    sg_nmm = [len(x) for x in mm_slots]
    NMM = sum(sg_nmm)
    NBLK = sum(sg_nblk)
    GCOLS = int(cap.sum()) * 8

    gidx_arrs, gslot_arrs, gval_arrs = [], [], []
    for c in range(NCORES):
        d = per_core[c]
        gidx = np.zeros((P, GCOLS), np.int16)
        gslot = np.full((P, NMM), -1.0, np.float32)
        gval = np.zeros((P, NBLK), np.float32)
        gcol = 0
        for s in range(NSG):
            for w in range(NWIN):
                k = s * NWIN + w
                ck = int(cap[k])
                if ck == 0:
                    continue
                st, n = run_starts[c, k], run_counts[c, k]
                gidx[:, gcol: gcol + ck * 8] = _pack_idx(
                    d["gpos"][st: st + n], ck)
                gcol += ck * 8
        # vals, block-major within sg
        blk0 = 0
        for s in range(NSG):
            for w in range(NWIN):
                k = s * NWIN + w
                st, n = run_starts[c, k], run_counts[c, k]
                for b in range(int(cap[k])):
                    lo, hi = b * P, min(b * P + P, int(n))
                    if lo < n:
                        gval[: hi - lo, blk0 + int(blk_base[k]) + b] = \
                            d["val"][st + lo: st + hi]
            blk0 += sg_nblk[s]
        # slots
        mi = 0
        for s in range(NSG):
            lt0 = SG_LIST[s][0]
            # map blk_in_sg -> (k, b)
            kb = {}
            for w in range(NWIN):
                k = s * NWIN + w
                for b in range(int(cap[k])):
                    kb[int(blk_base[k] + b)] = (k, b)
            for (bsg, tl) in mm_slots[s]:
                k, b = kb[bsg]
                st, n = run_starts[c, k], run_counts[c, k]
                lo, hi = b * P, min(b * P + P, int(n))
                if lo < n:
                    seg_t = d["ltile"][st + lo: st + hi]
                    seg_s = d["slot"][st + lo: st + hi]
                    sel = seg_t == (lt0 + tl)
                    col = np.full(P, -1.0, np.float32)
                    col[: hi - lo][sel] = seg_s[sel]
                    gslot[:, mi] = col
                mi += 1
        gidx_arrs.append(gidx)
        gslot_arrs.append(gslot.astype(np.float16))
        gval_arrs.append(gval.astype(np.float16))

    structure = (tuple(cap.tolist()),
                 tuple(x for sl in mm_slots for x in sl),
                 tuple(sg_nmm), tuple(sg_nblk))
    meta = dict(cap=cap, blk_base=blk_base, sg_nblk=sg_nblk, sg_nmm=sg_nmm,
                mm_slots=mm_slots, NMM=NMM, NBLK=NBLK, GCOLS=GCOLS,
                gidx_arrs=gidx_arrs, gslot_arrs=gslot_arrs,
                gval_arrs=gval_arrs)
    return structure, meta


def _build_program(structure, meta):
    import concourse.mybir as mybir
    import concourse.tile as tile
    from concourse import bacc
    from concourse.masks import make_identity

    f16 = mybir.dt.float16
    f32 = mybir.dt.float32
    i16 = mybir.dt.int16

    cap = meta["cap"]
    blk_base = meta["blk_base"]
    sg_nblk = meta["sg_nblk"]
    sg_nmm = meta["sg_nmm"]
    mm_slots = meta["mm_slots"]
    NMM = meta["NMM"]
    NBLK = meta["NBLK"]
    GCOLS = meta["GCOLS"]
    TOTBLK = max(sg_nblk)
    MAXMM = max(sg_nmm)

    nc = bacc.Bacc(None, num_devices=NCORES, num_swdge_queues=4)
    x0p16 = nc.dram_tensor("x0p16", [NPAD, D], f16, kind="ExternalInput")
    x0sh32 = nc.dram_tensor("x0sh32", [SHARD, D], f32, kind="ExternalInput")
    gidx = nc.dram_tensor("gidx", [P, GCOLS], i16, kind="ExternalInput")
    gslot = nc.dram_tensor("gslot", [P, NMM], f16, kind="ExternalInput")
    gval = nc.dram_tensor("gval", [P, NBLK], f16, kind="ExternalInput")
    w1 = nc.dram_tensor("w1", [D, D], f16, kind="ExternalInput")
    w2 = nc.dram_tensor("w2", [D, D], f16, kind="ExternalInput")
    b1 = nc.dram_tensor("b1", [D, 1], f32, kind="ExternalInput")
    b2 = nc.dram_tensor("b2", [D, 1], f32, kind="ExternalInput")
    nrm = nc.dram_tensor("nrm", [3, SHARD, D], f32, kind="ExternalOutput")

    with tile.TileContext(nc) as tc:
        with (
            tc.tile_pool(name="const", bufs=1) as cpool,
            tc.tile_pool(name="meta", bufs=1) as mpool,
            tc.tile_pool(name="gb", bufs=2) as gpool,
            tc.tile_pool(name="oh", bufs=3) as opool,
            tc.tile_pool(name="work", bufs=3) as wpool,
            tc.tile_pool(name="acc", bufs=3) as apool,
            tc.tile_pool(name="psy", bufs=3, space="PSUM") as psy,
            tc.tile_pool(name="psx", bufs=2, space="PSUM") as psx,
            tc.tile_pool(name="psz", bufs=3, space="PSUM") as psz,
            tc.tile_pool(name="dram", bufs=1, space="DRAM") as dram,
        ):
            ident = cpool.tile([P, P], f16)
            make_identity(nc, ident)
            iota_i = cpool.tile([P, P], mybir.dt.int32)
            nc.gpsimd.iota(iota_i, pattern=[[1, P]], base=0,
                           channel_multiplier=0)
            iota_h = cpool.tile([P, P], f16)
            nc.vector.tensor_copy(iota_h, iota_i)
            ones_h = cpool.tile([P, 1], f16)
            nc.vector.memset(ones_h, 1.0)
            OHC_C = (max(sg_nmm) + 4) // 5
            iota_rep = cpool.tile([P, OHC_C, P], f16)
            nc.vector.tensor_copy(
                iota_rep,
                iota_h[:, :].rearrange("p (o c) -> p o c", o=1).to_broadcast(
                    [P, OHC_C, P]))
            w1_t = cpool.tile([P, P], f16)
            nc.sync.dma_start(out=w1_t, in_=w1[:, :])
            w2_t = cpool.tile([P, P], f16)
            nc.sync.dma_start(out=w2_t, in_=w2[:, :])
            b1_t = cpool.tile([P, 1], f32)
            nc.sync.dma_start(out=b1_t, in_=b1[:, :])
            b2_t = cpool.tile([P, 1], f32)
            nc.sync.dma_start(out=b2_t, in_=b2[:, :])
            gidx_t = mpool.tile([P, GCOLS], i16)
            nc.sync.dma_start(out=gidx_t, in_=gidx[:, :])
            gslot_t = mpool.tile([P, NMM], f16)
            nc.sync.dma_start(out=gslot_t, in_=gslot[:, :])
            gval_t = mpool.tile([P, NBLK], f16)
            nc.sync.dma_start(out=gval_t, in_=gval[:, :])

            xsh = dram.tile([SHARD, D], f16)
            xgs = [dram.tile([LOC_PER_SEC[q] * P * NCORES, D], f16,
                             addr_space="Shared", name=f"xg{q}")
                   for q in range(4)]

            # hop 0: l2norm own shard of x0 (fp32), 4 tiles at a time
            for lt0 in range(0, NT_L, 4):
                nt = min(4, NT_L - lt0)
                x0c = wpool.tile([P, 4, D], f32, tag="x0c")
                nc.sync.dma_start(
                    out=x0c[:, :nt, :],
                    in_=x0sh32[lt0 * P: (lt0 + nt) * P, :].rearrange(
                        "(t p) d -> p t d", p=P))
                sq = wpool.tile([P, 4, D], f32, tag="h0sq")
                nc.vector.tensor_tensor(out=sq[:, :nt, :], in0=x0c[:, :nt, :],
                                        in1=x0c[:, :nt, :],
                                        op=mybir.AluOpType.mult)
                ss = wpool.tile([P, 4], f32, tag="h0ss")
                nc.vector.tensor_reduce(out=ss[:, :nt], in_=sq[:, :nt, :],
                                        axis=mybir.AxisListType.X,
                                        op=mybir.AluOpType.add)
                nr = wpool.tile([P, 4], f32, tag="h0nr")
                nc.scalar.sqrt(nr[:, :nt], ss[:, :nt])
                nc.vector.tensor_scalar_max(nr[:, :nt], nr[:, :nt], 1e-12)
                ri = wpool.tile([P, 4], f32, tag="h0ri")
                nc.vector.reciprocal(ri[:, :nt], nr[:, :nt])
                o0 = apool.tile([P, 4, D], f32, tag="h0out")
                nc.vector.tensor_tensor(
                    out=o0[:, :nt, :], in0=x0c[:, :nt, :],
                    in1=ri[:, :nt].rearrange("p (t o) -> p t o",
                                             o=1).to_broadcast([P, nt, D]),
                    op=mybir.AluOpType.mult)
                nc.sync.dma_start(
                    out=nrm[0, lt0 * P: (lt0 + nt) * P, :].rearrange(
                        "(t p) d -> p t d", p=P),
                    in_=o0[:, :nt, :])

            def graph_hop(hop, src, w_t, b_t, write_xsh):
                blk0 = 0
                mm0 = 0
                gcol_of = np.zeros(NSG * NWIN, np.int64)
                acc = 0
                for k in range(NSG * NWIN):
                    gcol_of[k] = acc
                    acc += int(cap[k]) * 8
                for s in range(NSG):
                    lt0, nt = SG_LIST[s]
                    nblk = sg_nblk[s]
                    nmm = sg_nmm[s]
                    gbuf = gpool.tile([P, TOTBLK, P], f16, tag="gbuf")
                    for w in range(NWIN):
                        k = s * NWIN + w
                        ck = int(cap[k])
                        if ck == 0:
                            continue
                        b0 = int(blk_base[k])
                        if isinstance(src, list):
                            q = min(w // 2, 3)
                            w0 = (w - 2 * q) * WIN
                            src_w = src[q][w0: w0 + WIN, :]
                        else:
                            src_w = src[w * WIN: (w + 1) * WIN, :]
                        nc.gpsimd.dma_gather(
                            gbuf[:, b0: b0 + ck, :],
                            src_w,
                            gidx_t[:, gcol_of[k]: gcol_of[k] + ck * 8],
                            num_idxs=ck * P, num_idxs_reg=ck * P,
                            elem_size=D, single_packet=False,
                            queue_num=1 + (s * NWIN + w) % 3,
                        )
                    # scale rows by edge vals (one batched op)
                    nc.vector.tensor_tensor(
                        out=gbuf[:, :nblk, :], in0=gbuf[:, :nblk, :],
                        in1=gval_t[:, blk0: blk0 + nblk].rearrange(
                            "p (b o) -> p b o", o=1).to_broadcast(
                            [P, nblk, P]),
                        op=mybir.AluOpType.mult)
                    # batched one-hot build, in 2 chunks to bound SBUF;
                    # slots are tile-major so each y accumulation is a
                    # single uninterrupted PSUM group.
                    nslots = [0] * nt
                    for (_, tl) in mm_slots[s]:
                        nslots[tl] += 1
                    seen = [0] * nt
                    y_ps = [None] * nt
                    OHC = (MAXMM + 4) // 5
                    oh = None
                    m1c = 0
                    for mi, (bsg, tl) in enumerate(mm_slots[s]):
                        if mi >= m1c:
                            m0c = mi
                            m1c = min(m0c + OHC, nmm)
                            nmmc = m1c - m0c
                            oh = opool.tile([P, OHC, P], f16, tag="oh")
                            nc.vector.tensor_tensor(
                                out=oh[:, :nmmc, :],
                                in0=iota_rep[:, :nmmc, :],
                                in1=gslot_t[:, mm0 + m0c: mm0 + m1c].rearrange(
                                    "p (m o) -> p m o", o=1).to_broadcast(
                                    [P, nmmc, P]),
                                op=mybir.AluOpType.is_equal)
                        if seen[tl] == 0:
                            y_ps[tl] = psy.tile([P, P], f32, space="PSUM",
                                                tag="y", name="y_ps")
                        seen[tl] += 1
                        nc.tensor.matmul(
                            y_ps[tl], lhsT=gbuf[:, bsg, :],
                            rhs=oh[:, mi - m0c, :],
                            start=(seen[tl] == 1),
                            stop=(seen[tl] == nslots[tl]))
                    # per-tile epilogue
                    xacc = apool.tile([P, 8, D], f16, tag="xacc")
                    oacc = apool.tile([P, 8, D], f32, tag="oacc")
                    for t in range(nt):
                        yT = wpool.tile([P, P], f16, tag="yT")
                        if nslots[t] == 0:
                            nc.vector.memset(yT, 0.0)
                        else:
                            nc.scalar.copy(yT, y_ps[t])
                        xn = psx.tile([P, 512], f32, space="PSUM", tag="xn")
                        x_ps = xn[:, :P]
                        nc.tensor.matmul(x_ps, lhsT=w_t, rhs=yT,
                                         start=True, stop=True)
                        xT = wpool.tile([P, P], f16, tag="xT")
                        nc.scalar.activation(
                            xT, x_ps, mybir.ActivationFunctionType.Identity,
                            bias=b_t[:, :1])
                        sqT = wpool.tile([P, P], f16, tag="sqT")
                        nc.scalar.activation(
                            sqT, x_ps, mybir.ActivationFunctionType.Square,
                            bias=b_t[:, :1])
                        n2_ps = xn[:, P: P + 1]
                        nc.tensor.matmul(n2_ps, lhsT=sqT, rhs=ones_h,
                                         start=True, stop=True)
                        z_ps = psz.tile([P, P], f16, space="PSUM", tag="z")
                        nc.tensor.transpose(z_ps, xT, ident)
                        nc.scalar.copy(xacc[:, t, :], z_ps)
                        nr = wpool.tile([P, 1], f32, tag="nr")
                        nc.scalar.sqrt(nr, n2_ps)
                        nc.vector.tensor_scalar_max(nr, nr, 1e-12)
                        ri = wpool.tile([P, 1], f32, tag="ri")
                        nc.vector.reciprocal(ri, nr)
                        nc.scalar.mul(oacc[:, t, :], xacc[:, t, :],
                                      ri[:, :1])
                    nc.sync.dma_start(
                        out=nrm[hop, lt0 * P: (lt0 + nt) * P, :].rearrange(
                            "(t p) d -> p t d", p=P),
                        in_=oacc[:, :nt, :])
                    if write_xsh:
                        nc.sync.dma_start(
                            out=xsh[lt0 * P: (lt0 + nt) * P, :].rearrange(
                                "(t p) d -> p t d", p=P),
                            in_=xacc[:, :nt, :])
                        for q in range(4):
                            if SEC_LAST_SG[q] == s:
                                r0 = LOC_BASE[q] * P
                                rn = LOC_PER_SEC[q] * P
                                g0 = SEC_ROWBASE[q]
                                gn = rn * NCORES
                                nc.gpsimd.collective_compute(
                                    "AllGather", mybir.AluOpType.bypass,
                                    replica_groups=[list(range(NCORES))],
                                    ins=[xsh[r0: r0 + rn, :].opt()],
                                    outs=[xgs[q][:, :].opt()],
                                )
                    blk0 += nblk
                    mm0 += nmm

            graph_hop(1, x0p16, w1_t, b1_t, True)
            graph_hop(2, xgs, w2_t, b2_t, False)

    nc.compile()
    return nc


def _install_ntff_shim():
    import types
    if "antenv.axon_hooks" in sys.modules:
        return
    mod = types.ModuleType("antenv.axon_hooks")
    mod._hook = None

    def set_axon_ntff_profile_hook(h):
        mod._hook = h

    def get_axon_ntff_profile_hook():
        return mod._hook

    mod.set_axon_ntff_profile_hook = set_axon_ntff_profile_hook
    mod.get_axon_ntff_profile_hook = get_axon_ntff_profile_hook
    sys.modules["antenv.axon_hooks"] = mod
    try:
        from trn_agent_boot.trn_boot import _ntff_profile_via_ctypes
        mod._hook = _ntff_profile_via_ctypes("/opt/axon/libaxon_pjrt.so")
    except Exception:
        mod._hook = None


def kernel(node_emb, attri_emb, W1, b1, W2, b2, edge_val,
           edge_row, edge_col, pos_src, pos_dst, neg_src, neg_dst):
    global LAST_RESULTS
    _install_ntff_shim()
    from concourse.bass_utils import run_bass_kernel_spmd

    node_emb, attri_emb, W1, b1, W2, b2 = [
        np.asarray(a) for a in (node_emb, attri_emb, W1, b1, W2, b2)]
    edge_val, edge_row, edge_col = [
        np.asarray(a) for a in (edge_val, edge_row, edge_col)]
    pos_src, pos_dst, neg_src, neg_dst = [
        np.asarray(a) for a in (pos_src, pos_dst, neg_src, neg_dst)]

    structure, meta = _prep(edge_row, edge_col, edge_val)

    import time as _time
    if structure in _CACHE:
        nc = _CACHE[structure]
    else:
        t0 = _time.time()
        nc = _build_program(structure, meta)
        print(f"[kernel] build+schedule: {_time.time() - t0:.1f}s, "
              f"{len(nc.inst_map)} instructions", flush=True)
        _CACHE[structure] = nc

    x0 = np.concatenate([node_emb, attri_emb], axis=0).astype(np.float32)
    x0pad = np.zeros((NPAD, D), np.float32)
    x0pad[:N] = x0
    pp = np.asarray(_perm_pos(np.arange(NPAD)))
    x0perm = np.zeros((NPAD, D), np.float32)
    x0perm[pp] = x0pad
    x0perm16 = x0perm.astype(np.float16)

    in_maps = []
    for c in range(NCORES):
        # core c's shard rows in local-tile order, from the permuted layout
        sl = []
        for q in range(4):
            b = SEC_ROWBASE[q] + c * LOC_PER_SEC[q] * P
            sl.append(x0perm[b: b + LOC_PER_SEC[q] * P])
        in_maps.append({
            "x0p16": x0perm16,
            "x0sh32": np.concatenate(sl, axis=0),
            "gidx": meta["gidx_arrs"][c],
            "gslot": meta["gslot_arrs"][c],
            "gval": meta["gval_arrs"][c],
            "w1": W1.astype(np.float16),
            "w2": W2.astype(np.float16),
            "b1": b1.reshape(D, 1).astype(np.float32),
            "b2": b2.reshape(D, 1).astype(np.float32),
        })

    trace = os.environ.get("BASS_GNN_TRACE", "0") == "1"
    t0 = _time.time()
    res = run_bass_kernel_spmd(nc, in_maps, core_ids=list(range(NCORES)),
                               trace=trace)
    print(f"[kernel] compile+run: {_time.time() - t0:.1f}s", flush=True)
    LAST_RESULTS = res

    # host-side pair expansion from dense normalized shards
    nrm_all = np.stack([res.results[c]["nrm"] for c in range(NCORES)])
    # global row r -> (owner, local row)
    out = np.zeros((4, 3, E_PAIR, D), np.float32)
    streams = [pos_src, pos_dst, neg_src, neg_dst]
    for st, idx in enumerate(streams):
        r = idx.astype(np.int64)
        t = r >> 7
        own = t % NCORES
        lr = np.asarray(_local_tile(t)) * P + (r & 127)
        for h in range(3):
            out[st, h] = nrm_all[own, h, lr]
    return out
